# revision 3
# baseline (speedup 1.0000x reference)
"""Trainium2 Bass kernel for nn_LossKMeansWasserstein — single-launch design.

Architecture (v2): wall-clock in this axon-tunneled environment is dominated
by per-launch overhead (~0.35s stock, ~0.08s with a cached-jit launcher) and
host->device transfer (~50MB/s), not device compute (<1ms). So:

  1. ONE device launch per call (no cost-max prepass): eps0 per cost kind is
     replaced by the upper bound 0.5*(max|x|+max|y|)^2 (sim: <2e-4 effect on
     the loss at NITER=22 vs the reference's exact-max schedule).
  2. Cluster k lives entirely on core k: its 3 Sinkhorn problems (xy, xx, yy)
     run interleaved for cross-problem engine pipelining.
  3. Uploads are compact (~0.5MB/core): augmented point tiles + tiny scalar
     tables. Big operand tiles (moving side with the dynamic h-row) are built
     on device; per-(t,problem) eps scalars are broadcast to 128 partitions
     with a single ones-matmul.
  4. The jitted PJRT launcher is built once and cached; per-call overhead is
     concat + dispatch only.

Math: log-domain Sinkhorn on tilde-potentials G~ = g - 0.5|y|^2. The PE
computes V_ij = h_j + x_i.y_j - 0.5|x_i|^2 in one matmul per 128-row block
(h rides row 64 of the moving tile, -0.5|x|^2 rides row 65 of the stationary
tile). Row-max on DVE, fused exp+row-sum on ACT, then q = lnS + m'/eps is
transposed via PE and written back (scaled by -eps, biased by eps*log w) as
the next half-update's h-row.
"""
import os
import sys
from contextlib import ExitStack

import numpy as np

sys.path.insert(0, "/opt/trn_rl_repo")

import concourse.bass as bass  # noqa: E402
import concourse.tile as tile  # noqa: E402
from concourse import bacc, mybir, bass2jax  # noqa: E402
from concourse.masks import make_identity  # noqa: E402

import jax  # noqa: E402
from jax.sharding import Mesh, PartitionSpec  # noqa: E402

from jax.experimental.shard_map import shard_map as _sm  # noqa: E402


def _shard_map(f, mesh, in_specs, out_specs, check_rep):
    return _sm(f, mesh=mesh, in_specs=in_specs, out_specs=out_specs,
               check_rep=check_rep)

F32 = mybir.dt.float32
AF = mybir.ActivationFunctionType
ALU = mybir.AluOpType

N, M, D, K = 3072, 3072, 64, 8
BLUR = 0.05
EPS = np.float32(BLUR ** 2)
SCAL2 = np.float32(0.8 ** 2)
NITER = int(os.environ.get("KM_NITER", "22"))
NSEQ = NITER + 1
BIG = np.float32(1e7)
NCORES = 8

_cache = {}


def _ceil128(v):
    return max(128, ((v + 127) // 128) * 128)


# --------------------------------------------------------------------------
# device kernel
# --------------------------------------------------------------------------

def _build(S):
    NB = S // 128
    nc = bacc.Bacc("TRN2", target_bir_lowering=False, debug=False,
                   num_devices=NCORES)

    d = {}
    for name, shape in (
        ("xstat", [66, S]), ("ystat", [66, S]), ("inith", [4, S]),
        ("halfnx", [128, NB]), ("halfny", [128, NB]),
        ("aw", [128, NB]), ("bw", [128, NB]),
        ("bscal", [1, 12 * NSEQ]), ("rtab", [1, 3 * NSEQ]),
        ("caug", [66, 8]),
    ):
        d[name] = nc.dram_tensor(name, shape, F32, kind="ExternalInput").ap()
    OC = 6 * NB + 1
    d_out = nc.dram_tensor("osum", [16, OC], F32, kind="ExternalOutput").ap()

    with tile.TileContext(nc) as tc, ExitStack() as ctx:
        cpool = ctx.enter_context(tc.tile_pool(name="cpool", bufs=1))
        g = {}
        for nm in ("xstat", "ystat", "inith", "halfnx", "halfny", "aw", "bw",
                   "bscal", "rtab", "caug"):
            t = cpool.tile(list(d[nm].shape), F32, tag=f"in_{nm}")
            nc.sync.dma_start(t[:], d[nm][:])
            g[nm] = t
        ident = cpool.tile([128, 128], F32, tag="ident")
        make_identity(nc, ident[:])
        g["ident"] = ident

        osum = cpool.tile([128, OC], F32)
        nc.vector.memset(osum[:], 0.0)

        # ---- moving-side tiles (pts rows + dynamic h row + ones row) ----
        movs = {}
        for nm, src, hrow in (("mov_y_xy", "ystat", 0), ("mov_x_xy", "xstat", None),
                              ("mov_x_xx_b", "xstat", 1), ("mov_x_xx_a", "xstat", None),
                              ("mov_y_yy_b", "ystat", 2), ("mov_y_yy_a", "ystat", None)):
            mt = cpool.tile([66, S], F32, tag=nm)
            nc.sync.dma_start(mt[0:64, :], g[src][0:64, :])
            nc.sync.dma_start(mt[65:66, :], g["inith"][3:4, :])
            if hrow is None:
                nc.vector.memset(mt[64:65, :], 0.0)
            else:
                nc.sync.dma_start(mt[64:65, :], g["inith"][hrow:hrow + 1, :])
            movs[nm] = mt

        # ---- broadcast per-(problem,t) scalars to 128 partitions ----
        onesrow = cpool.tile([1, 128], F32)
        nc.vector.memset(onesrow[:], 1.0)
        btab = cpool.tile([128, 12 * NSEQ], F32)
        with tc.tile_pool(name="setup_ps", bufs=1, space="PSUM") as sps:
            bc = sps.tile([128, 12 * NSEQ], F32, tag="bc")
            nc.tensor.matmul(bc[:], onesrow[:], g["bscal"][:])
            nc.scalar.copy(btab[:], bc[:])

            # ---- filling partial sums (independent of sinkhorn) ----
            fillps = sps.tile([8, 1], F32, tag="fillps")
            for b in range(NB):
                dxp = sps.tile([128, 8], F32, tag="dxp")
                nc.tensor.matmul(dxp[:], g["xstat"][:, b * 128:(b + 1) * 128],
                                 g["caug"][:])
                mind = cpool.tile([128, 1], F32, tag="mind")
                nc.vector.tensor_reduce(mind[:], dxp[:], mybir.AxisListType.X,
                                        ALU.min)
                et = cpool.tile([128, 8], F32, tag="et")
                ssum = cpool.tile([128, 1], F32, tag="ssum")
                nc.scalar.activation(et[:], dxp[:], AF.Exp, bias=mind[:],
                                     scale=-1.0, accum_out=ssum[:])
                rs = cpool.tile([128, 1], F32, tag="rs")
                nc.vector.reciprocal(rs[:], ssum[:])
                soft = cpool.tile([128, 8], F32, tag="soft")
                nc.vector.tensor_scalar_mul(soft[:], et[:], rs[:])
                nc.tensor.matmul(fillps[:], soft[:], g["aw"][:, b:b + 1],
                                 start=(b == 0), stop=(b == NB - 1))
            nc.scalar.copy(osum[0:8, OC - 1:OC], fillps[:])

        # ---- the 3 sinkhorn problems, interleaved ----
        probs = [
            dict(pi=0, statA=g["xstat"], statB=g["ystat"],
                 movA=movs["mov_x_xy"], movB=movs["mov_y_xy"],
                 hA=g["halfnx"], hB=g["halfny"], wA=g["aw"], wB=g["bw"]),
            dict(pi=1, statA=g["xstat"], statB=g["xstat"],
                 movA=movs["mov_x_xx_a"], movB=movs["mov_x_xx_b"],
                 hA=g["halfnx"], hB=g["halfnx"], wA=g["aw"], wB=g["aw"]),
            dict(pi=2, statA=g["ystat"], statB=g["ystat"],
                 movA=movs["mov_y_yy_a"], movB=movs["mov_y_yy_b"],
                 hA=g["halfny"], hB=g["halfny"], wA=g["bw"], wB=g["bw"]),
        ]

        psv = ctx.enter_context(tc.tile_pool(name="psv", bufs=2, space="PSUM"))
        psq = ctx.enter_context(tc.tile_pool(name="psq", bufs=1, space="PSUM"))
        wpool = ctx.enter_context(tc.tile_pool(name="wpool", bufs=2))
        epool = ctx.enter_context(tc.tile_pool(name="epool", bufs=1))

        def half(pr, t, fside, final):
            pi = pr["pi"]
            if fside:
                stat, mov_in, mov_out = pr["statA"], pr["movB"], pr["movA"]
                halfn, w = pr["hA"], pr["wA"]
                # h' consumed by the g-half of the SAME iteration t
                tq_off = (6 + pi) * NSEQ + t        # -logw_A
            else:
                stat, mov_in, mov_out = pr["statB"], pr["movA"], pr["movB"]
                halfn, w = pr["hB"], pr["wB"]
                # h' consumed by the f-half of iteration t+1 (incl. final)
                tq_off = (9 + pi) * NSEQ + t        # -(eps_{t+1}/eps_t)*logw_B
            inveps = g["btab_view"][:, pi * NSEQ + t:pi * NSEQ + t + 1]
            nginveps = g["btab_view"][:, (3 + pi) * NSEQ + t:
                                      (3 + pi) * NSEQ + t + 1]
            ne_off = pi * NSEQ + t                  # -eps_t

            sd = "f" if fside else "g"
            lnm = wpool.tile([128, 2 * NB], F32, tag=f"lnm{pi}{sd}")
            sv = wpool.tile([128, NB], F32, tag=f"sv{pi}{sd}")
            for b in range(NB):
                vps = psv.tile([128, S], F32, tag="vps")
                for c0 in range(0, S, 512):
                    c1 = min(c0 + 512, S)
                    nc.tensor.matmul(vps[:, c0:c1],
                                     stat[:, b * 128:(b + 1) * 128],
                                     mov_in[:, c0:c1])
                nc.vector.tensor_reduce(lnm[:, NB + b:NB + b + 1], vps[:],
                                        mybir.AxisListType.X, ALU.max)
                bv = wpool.tile([128, 1], F32, tag=f"bv{pi}")
                nc.vector.tensor_scalar_mul(bv[:], lnm[:, NB + b:NB + b + 1],
                                            nginveps)
                expo = epool.tile([128, S], F32, tag=f"expo{pi}")
                nc.scalar.activation(expo[:], vps[:], AF.Exp, bias=bv[:],
                                     scale=inveps, accum_out=sv[:, b:b + 1])
            nc.scalar.activation(lnm[:, 0:NB], sv[:], AF.Ln)
            nc.vector.tensor_add(lnm[:, NB:2 * NB], lnm[:, NB:2 * NB],
                                 halfn[:])
            if final:
                q = pi * 2 + (0 if fside else 1)
                dps = psq.tile([2 * NB, NB], F32, tag="dot")
                nc.tensor.matmul(dps[:], lnm[:], w[:])
                nc.scalar.copy(osum[0:2 * NB, q * NB:(q + 1) * NB], dps[:])
            else:
                tq = g["btab_view"][:, tq_off:tq_off + 1]
                qv = wpool.tile([128, NB], F32, tag=f"qv{pi}")
                nc.vector.tensor_scalar_mul(qv[:], lnm[:, NB:2 * NB], inveps)
                nc.vector.tensor_add(qv[:], qv[:], lnm[:, 0:NB])
                nc.vector.tensor_scalar_add(qv[:], qv[:], tq)
                qT = psq.tile([1, S], F32, tag="qT")
                for b in range(NB):
                    nc.tensor.matmul(qT[0:1, b * 128:(b + 1) * 128],
                                     qv[:, b:b + 1], g["ident"][:])
                nc.scalar.activation(
                    mov_out[64:65, :], qT[:], AF.Copy, bias=0.0,
                    scale=g["rtab"][0:1, ne_off:ne_off + 1])

        g["btab_view"] = btab
        for t in range(NITER):
            for pr in probs:
                half(pr, t, True, False)
            for pr in probs:
                half(pr, t, False, False)
        for pr in probs:
            half(pr, NITER, True, True)
        for pr in probs:
            half(pr, NITER, False, True)

        nc.sync.dma_start(d_out[:], osum[0:16, :])
    nc.compile()
    return nc


# --------------------------------------------------------------------------
# cached-jit PJRT launcher (per-call jax.jit in run_bass_kernel_spmd costs
# ~0.3s of retracing; build the jitted callable once instead)
# --------------------------------------------------------------------------

def _make_runner(nc):
    bass2jax.install_neuronx_cc_hook()
    partition_name = (nc.partition_id_tensor.name
                      if nc.partition_id_tensor else None)
    in_names, out_names, out_avals, zero_shapes = [], [], [], []
    for alloc in nc.m.functions[0].allocations:
        if not isinstance(alloc, mybir.MemoryLocationSet):
            continue
        name = alloc.memorylocations[0].name
        if alloc.kind == "ExternalInput":
            if name != partition_name:
                in_names.append(name)
        elif alloc.kind == "ExternalOutput":
            shape = tuple(alloc.tensor_shape)
            dtype = mybir.dt.np(alloc.dtype)
            out_names.append(name)
            out_avals.append(jax.core.ShapedArray(shape, dtype))
            zero_shapes.append((shape, dtype))
    n_params = len(in_names)
    n_outs = len(out_avals)
    in_names_all = list(in_names) + list(out_names)
    if partition_name is not None:
        in_names_all.append(partition_name)
    donate = tuple(range(n_params, n_params + n_outs))

    def _body(*args):
        operands = list(args)
        if partition_name is not None:
            operands.append(bass2jax.partition_id_tensor())
        outs = bass2jax._bass_exec_p.bind(
            *operands, out_avals=tuple(out_avals),
            in_names=tuple(in_names_all), out_names=tuple(out_names),
            lowering_input_output_aliases=(), sim_require_finite=True,
            sim_require_nnan=True, nc=nc)
        return tuple(outs)

    devices = jax.devices()[:NCORES]
    mesh = Mesh(np.asarray(devices), ("core",))
    in_specs = (PartitionSpec("core"),) * (n_params + n_outs)
    out_specs = (PartitionSpec("core"),) * n_outs
    sharded = jax.jit(
        _shard_map(_body, mesh, in_specs, out_specs, False),
        donate_argnums=donate, keep_unused=True)

    def run(in_maps):
        concat_in = [
            np.concatenate([np.asarray(in_maps[c][nm]) for c in range(NCORES)],
                           axis=0)
            for nm in in_names]
        concat_zeros = [np.zeros((NCORES * s[0], *s[1:]), dt)
                        for s, dt in zero_shapes]
        out_arrs = sharded(*concat_in, *concat_zeros)
        return [
            {nm: np.asarray(out_arrs[i]).reshape(NCORES, *out_avals[i].shape)[c]
             for i, nm in enumerate(out_names)}
            for c in range(NCORES)]

    return run


# --------------------------------------------------------------------------
# host orchestration
# --------------------------------------------------------------------------

def _pk(vec, nb):
    """[nb*128] -> [128, nb]; column b holds points b*128..b*128+127."""
    return np.ascontiguousarray(vec.reshape(nb, 128).T)


def kernel(x, target, cluster_centers, filling_target, prediction_target):
    f32 = np.float32
    x = np.asarray(x, f32)
    y = np.asarray(target, f32)
    cc = np.asarray(cluster_centers, f32)
    filling_target = np.asarray(filling_target, f32)
    pt = np.asarray(prediction_target)

    nx = (x * x).sum(-1).astype(f32)
    ny = (y * y).sum(-1).astype(f32)
    ncc = (cc * cc).sum(-1).astype(f32)
    d_x = (nx[:, None] + ncc[None, :] - 2.0 * (x @ cc.T)).astype(f32)
    pred_x = d_x.argmin(1)

    idx_x = [np.where(pred_x == k)[0] for k in range(K)]
    idx_y = [np.where(pt == k)[0] for k in range(K)]
    nk = [len(i) for i in idx_x]
    mk = [len(i) for i in idx_y]
    S = _ceil128(max(max(nk), max(mk)))
    NB = S // 128
    OC = 6 * NB + 1

    # eps0 upper bounds per cost kind (exact max of C is not worth a launch)
    mx = np.sqrt(nx.max())
    my = np.sqrt(ny.max())
    eps0 = {"xy": max(f32(0.5 * (mx + my) ** 2), EPS),
            "xx": max(f32(0.5 * (2 * mx) ** 2), EPS),
            "yy": max(f32(0.5 * (2 * my) ** 2), EPS)}

    key = (S, NITER)
    if key not in _cache:
        nc = _build(S)
        _cache[key] = (nc, _make_runner(nc))
    nc, runner = _cache[key]

    t_arr = np.arange(NITER, dtype=f32)
    eps_seq = {}
    for kind, e0 in eps0.items():
        s = np.maximum(e0 * SCAL2 ** t_arr, EPS).astype(f32)
        eps_seq[kind] = np.concatenate([s, [EPS]]).astype(f32)
    kinds = ("xy", "xx", "yy")

    caug = np.zeros((66, 8), f32)
    caug[0:64] = -2.0 * cc.T
    caug[64] = ncc
    caug[65] = -2.0

    in_maps = []
    host_terms = np.zeros(NCORES, f32)   # sum_p coeff * (aw.halfnx + bw.halfny)
    valid = np.zeros((NCORES, 3), f32)
    coeffs = np.array([1.0, -0.5, -0.5], f32)

    for k in range(K):
        xk = x[idx_x[k]]
        yk = y[idx_y[k]]
        cx, cy = nk[k], mk[k]
        nxk = nx[idx_x[k]]
        nyk = ny[idx_y[k]]

        def stat_tile(pts, n2):
            t = np.zeros((66, S), f32)
            t[0:64, :pts.shape[0]] = pts.T
            t[64] = 1.0
            t[65, :pts.shape[0]] = -0.5 * n2
            return t

        xstat = stat_tile(xk, nxk)
        ystat = stat_tile(yk, nyk)

        lwx = f32(np.log(np.float64(1.0 / cx))) if cx else f32(0.0)
        lwy = f32(np.log(np.float64(1.0 / cy))) if cy else f32(0.0)
        # logw of the A (x/rows) and B (y/cols) side per problem
        lwA = (lwx, lwx, lwy)
        lwB = (lwy, lwx, lwy)

        inith = np.full((4, S), -BIG, f32)
        inith[0, :cy] = eps_seq["xy"][0] * lwy - 0.5 * nyk
        inith[1, :cx] = eps_seq["xx"][0] * lwx - 0.5 * nxk
        inith[2, :cy] = eps_seq["yy"][0] * lwy - 0.5 * nyk
        inith[3, :] = 1.0          # the constant ones row of the mov tiles

        hx = np.full(S, BIG, f32)
        hx[:cx] = 0.5 * nxk
        hy = np.full(S, BIG, f32)
        hy[:cy] = 0.5 * nyk
        awv = np.zeros(S, f32)
        if cx:
            awv[:cx] = f32(1.0 / cx)
        bwv = np.zeros(S, f32)
        if cy:
            bwv[:cy] = f32(1.0 / cy)

        bscal = np.zeros((1, 12 * NSEQ), f32)
        rtab = np.zeros((1, 3 * NSEQ), f32)
        for p, kind in enumerate(kinds):
            es = eps_seq[kind]
            bscal[0, p * NSEQ:(p + 1) * NSEQ] = 1.0 / es
            bscal[0, (3 + p) * NSEQ:(4 + p) * NSEQ] = -1.0 / es
            # tq tables: q += tq before the -eps_t-scaled writeback, so that
            # h' = eps_cons*logw - eps_t*q. f-side: cons = eps_t; g-side:
            # cons = eps_{t+1} (the f-half of the next iteration).
            bscal[0, (6 + p) * NSEQ:(7 + p) * NSEQ] = -lwA[p]
            tqg = np.zeros(NSEQ, f32)
            tqg[:NITER] = -(es[1:] / es[:NITER]) * lwB[p]
            bscal[0, (9 + p) * NSEQ:(10 + p) * NSEQ] = tqg
            rtab[0, p * NSEQ:(p + 1) * NSEQ] = -es

        in_maps.append({
            "xstat": xstat, "ystat": ystat, "inith": inith,
            "halfnx": _pk(hx, NB), "halfny": _pk(hy, NB),
            "aw": _pk(awv, NB), "bw": _pk(bwv, NB),
            "bscal": bscal, "rtab": rtab, "caug": caug,
        })
        vk = f32(1.0) if (cx > 0 and cy > 0) else f32(0.0)
        valid[k] = vk
        ha = f32((awv * hx).sum(dtype=np.float64)) if cx else f32(0.0)
        hb = f32((bwv * hy).sum(dtype=np.float64)) if cy else f32(0.0)
        # per problem p: f-side host const uses A weights, g-side B weights
        hostA = (ha, ha, hb)
        hostB = (hb, ha, hb)
        # g2 consumes the t=NITER-1 f-half's h-row, whose logw bias used
        # eps_{NITER-1} instead of EPS; the resulting potential is uniformly
        # shifted by -(eps_{NITER-1}-EPS)*logw_A — add the exact shift back.
        delta = [float(eps_seq[kinds[p]][NITER - 1] - EPS) * float(lwA[p])
                 for p in range(3)]
        host_terms[k] = vk * float(
            sum(coeffs[p] * (hostA[p] + hostB[p] + delta[p])
                for p in range(3)))

    results = runner(in_maps)

    loss_med = np.float64(0.0)
    fill = np.zeros(8, np.float64)
    for k in range(K):
        o = results[k]["osum"].astype(np.float64)
        fill += nk[k] * o[0:8, OC - 1]
        for p in range(3):
            s_p = 0.0
            for side in range(2):
                q = p * 2 + side
                blk = o[0:2 * NB, q * NB:(q + 1) * NB]
                dln = sum(blk[b, b] for b in range(NB))
                dmp = sum(blk[NB + b, b] for b in range(NB))
                s_p += -float(EPS) * dln - dmp
            loss_med += valid[k, p] * coeffs[p] * s_p
        loss_med += host_terms[k]

    filling_x = (fill / N).astype(f32)
    loss_fil = np.mean((filling_x - filling_target) ** 2, dtype=f32)
    return np.asarray(f32(loss_fil + f32(loss_med)))


# revision 4
# speedup vs baseline: 1.1284x; 1.1284x over previous
"""Trainium2 Bass kernel for nn_LossKMeansWasserstein — single-launch design.

Architecture (v2): wall-clock in this axon-tunneled environment is dominated
by per-launch overhead (~0.35s stock, ~0.08s with a cached-jit launcher) and
host->device transfer (~50MB/s), not device compute (<1ms). So:

  1. ONE device launch per call (no cost-max prepass): eps0 per cost kind is
     replaced by the upper bound 0.5*(max|x|+max|y|)^2 (sim: <2e-4 effect on
     the loss at NITER=22 vs the reference's exact-max schedule).
  2. Cluster k lives entirely on core k: its 3 Sinkhorn problems (xy, xx, yy)
     run interleaved for cross-problem engine pipelining.
  3. Uploads are compact (~0.5MB/core): augmented point tiles + tiny scalar
     tables. Big operand tiles (moving side with the dynamic h-row) are built
     on device; per-(t,problem) eps scalars are broadcast to 128 partitions
     with a single ones-matmul.
  4. The jitted PJRT launcher is built once and cached; per-call overhead is
     concat + dispatch only.

Math: log-domain Sinkhorn on tilde-potentials G~ = g - 0.5|y|^2. The PE
computes V_ij = h_j + x_i.y_j - 0.5|x_i|^2 in one matmul per 128-row block
(h rides row 64 of the moving tile, -0.5|x|^2 rides row 65 of the stationary
tile). Row-max on DVE, fused exp+row-sum on ACT, then q = lnS + m'/eps is
transposed via PE and written back (scaled by -eps, biased by eps*log w) as
the next half-update's h-row.
"""
import os
import sys
from contextlib import ExitStack

import numpy as np

sys.path.insert(0, "/opt/trn_rl_repo")

import concourse.bass as bass  # noqa: E402
import concourse.tile as tile  # noqa: E402
from concourse import bacc, mybir, bass2jax  # noqa: E402
from concourse.masks import make_identity  # noqa: E402

import jax  # noqa: E402
from jax.sharding import Mesh, PartitionSpec  # noqa: E402

from jax.experimental.shard_map import shard_map as _sm  # noqa: E402


def _shard_map(f, mesh, in_specs, out_specs, check_rep):
    return _sm(f, mesh=mesh, in_specs=in_specs, out_specs=out_specs,
               check_rep=check_rep)

F32 = mybir.dt.float32
BF16 = mybir.dt.bfloat16
AF = mybir.ActivationFunctionType
ALU = mybir.AluOpType

N, M, D, K = 3072, 3072, 64, 8
BLUR = 0.05
EPS = np.float32(BLUR ** 2)
SCAL2 = np.float32(0.8 ** 2)
NITER = int(os.environ.get("KM_NITER", "22"))
NSEQ = NITER + 1
BIG = np.float32(1e7)
NCORES = 8

_cache = {}


def _ceil128(v):
    return max(128, ((v + 127) // 128) * 128)


# --------------------------------------------------------------------------
# device kernel
# --------------------------------------------------------------------------

def _build(S):
    NB = S // 128
    nc = bacc.Bacc("TRN2", target_bir_lowering=False, debug=False,
                   num_devices=NCORES)

    d = {}
    for name, shape, dt in (
        ("xpts", [64, S], BF16), ("ypts", [64, S], BF16),
        ("xext", [2, S], F32), ("yext", [2, S], F32),
        ("inith", [4, S], F32),
        ("halfnx", [128, NB], F32), ("halfny", [128, NB], F32),
        ("aw", [128, NB], F32), ("bw", [128, NB], F32),
        ("bscal", [1, 12 * NSEQ], F32), ("rtab", [1, 3 * NSEQ], F32),
        ("cpts", [64, 8], BF16), ("cext", [2, 8], F32),
    ):
        d[name] = nc.dram_tensor(name, shape, dt, kind="ExternalInput").ap()
    OC = 6 * NB + 1
    d_out = nc.dram_tensor("osum", [16, OC], F32, kind="ExternalOutput").ap()

    with tile.TileContext(nc) as tc, ExitStack() as ctx:
        cpool = ctx.enter_context(tc.tile_pool(name="cpool", bufs=1))
        g = {}
        for nm in ("xpts", "ypts", "xext", "yext", "inith", "halfnx",
                   "halfny", "aw", "bw", "bscal", "rtab", "cpts", "cext"):
            t = cpool.tile(list(d[nm].shape), d[nm].tensor.dtype,
                           tag=f"in_{nm}")
            nc.sync.dma_start(t[:], d[nm][:])
            g[nm] = t
        ident = cpool.tile([128, 128], F32, tag="ident")
        make_identity(nc, ident[:])
        g["ident"] = ident

        osum = cpool.tile([128, OC], F32)
        nc.vector.memset(osum[:], 0.0)

        # ---- dynamic-row tiles: row 0 = h (rewritten each half), row 1 = 1 ----
        dyns = {}
        for nm, hrow in (("dyn_b_xy", 0), ("dyn_a_xy", None),
                         ("dyn_b_xx", 1), ("dyn_a_xx", None),
                         ("dyn_b_yy", 2), ("dyn_a_yy", None)):
            dt_ = cpool.tile([2, S], F32, tag=nm)
            nc.sync.dma_start(dt_[1:2, :], g["inith"][3:4, :])
            if hrow is None:
                nc.vector.memset(dt_[0:1, :], 0.0)
            else:
                nc.sync.dma_start(dt_[0:1, :], g["inith"][hrow:hrow + 1, :])
            dyns[nm] = dt_

        # ---- broadcast per-(problem,t) scalars to 128 partitions ----
        onesrow = cpool.tile([1, 128], F32)
        nc.vector.memset(onesrow[:], 1.0)
        btab = cpool.tile([128, 12 * NSEQ], F32)
        with tc.tile_pool(name="setup_ps", bufs=1, space="PSUM") as sps:
            bc = sps.tile([128, 12 * NSEQ], F32, tag="bc")
            nc.tensor.matmul(bc[:], onesrow[:], g["bscal"][:])
            nc.scalar.copy(btab[:], bc[:])

            # ---- filling partial sums (independent of sinkhorn) ----
            fillps = sps.tile([8, 1], F32, tag="fillps")
            for b in range(NB):
                dxp = sps.tile([128, 8], F32, tag="dxp")
                nc.tensor.matmul(dxp[:], g["xpts"][:, b * 128:(b + 1) * 128],
                                 g["cpts"][:], start=True, stop=False)
                nc.tensor.matmul(dxp[:], g["xext"][:, b * 128:(b + 1) * 128],
                                 g["cext"][:], start=False, stop=True)
                mind = cpool.tile([128, 1], F32, tag="mind")
                nc.vector.tensor_reduce(mind[:], dxp[:], mybir.AxisListType.X,
                                        ALU.min)
                et = cpool.tile([128, 8], F32, tag="et")
                ssum = cpool.tile([128, 1], F32, tag="ssum")
                nc.scalar.activation(et[:], dxp[:], AF.Exp, bias=mind[:],
                                     scale=-1.0, accum_out=ssum[:])
                rs = cpool.tile([128, 1], F32, tag="rs")
                nc.vector.reciprocal(rs[:], ssum[:])
                soft = cpool.tile([128, 8], F32, tag="soft")
                nc.vector.tensor_scalar_mul(soft[:], et[:], rs[:])
                nc.tensor.matmul(fillps[:], soft[:], g["aw"][:, b:b + 1],
                                 start=(b == 0), stop=(b == NB - 1))
            nc.scalar.copy(osum[0:8, OC - 1:OC], fillps[:])

        # ---- the 3 sinkhorn problems, interleaved ----
        probs = [
            dict(pi=0, ptsA=g["xpts"], ptsB=g["ypts"],
                 extA=g["xext"], extB=g["yext"],
                 dynA=dyns["dyn_a_xy"], dynB=dyns["dyn_b_xy"],
                 hA=g["halfnx"], hB=g["halfny"], wA=g["aw"], wB=g["bw"]),
            dict(pi=1, ptsA=g["xpts"], ptsB=g["xpts"],
                 extA=g["xext"], extB=g["xext"],
                 dynA=dyns["dyn_a_xx"], dynB=dyns["dyn_b_xx"],
                 hA=g["halfnx"], hB=g["halfnx"], wA=g["aw"], wB=g["aw"]),
            dict(pi=2, ptsA=g["ypts"], ptsB=g["ypts"],
                 extA=g["yext"], extB=g["yext"],
                 dynA=dyns["dyn_a_yy"], dynB=dyns["dyn_b_yy"],
                 hA=g["halfny"], hB=g["halfny"], wA=g["bw"], wB=g["bw"]),
        ]

        psv = ctx.enter_context(tc.tile_pool(name="psv", bufs=2, space="PSUM"))
        psq = ctx.enter_context(tc.tile_pool(name="psq", bufs=1, space="PSUM"))
        wpool = ctx.enter_context(tc.tile_pool(name="wpool", bufs=2))
        epool = ctx.enter_context(tc.tile_pool(name="epool", bufs=1))

        def half(pr, t, fside, final):
            pi = pr["pi"]
            if fside:
                ptsS, ptsM = pr["ptsA"], pr["ptsB"]
                extS, dyn_in, dyn_out = pr["extA"], pr["dynB"], pr["dynA"]
                halfn, w = pr["hA"], pr["wA"]
                # h' consumed by the g-half of the SAME iteration t
                tq_off = (6 + pi) * NSEQ + t        # -logw_A
            else:
                ptsS, ptsM = pr["ptsB"], pr["ptsA"]
                extS, dyn_in, dyn_out = pr["extB"], pr["dynA"], pr["dynB"]
                halfn, w = pr["hB"], pr["wB"]
                # h' consumed by the f-half of iteration t+1 (incl. final)
                tq_off = (9 + pi) * NSEQ + t        # -(eps_{t+1}/eps_t)*logw_B
            inveps = g["btab_view"][:, pi * NSEQ + t:pi * NSEQ + t + 1]
            nginveps = g["btab_view"][:, (3 + pi) * NSEQ + t:
                                      (3 + pi) * NSEQ + t + 1]
            ne_off = pi * NSEQ + t                  # -eps_t

            sd = "f" if fside else "g"
            lnm = wpool.tile([128, 2 * NB], F32, tag=f"lnm{pi}{sd}")
            sv = wpool.tile([128, NB], F32, tag=f"sv{pi}{sd}")
            for b in range(NB):
                vps = psv.tile([128, S], F32, tag="vps")
                for c0 in range(0, S, 512):
                    c1 = min(c0 + 512, S)
                    nc.tensor.matmul(vps[:, c0:c1],
                                     ptsS[:, b * 128:(b + 1) * 128],
                                     ptsM[:, c0:c1], start=True, stop=False)
                    nc.tensor.matmul(vps[:, c0:c1],
                                     extS[:, b * 128:(b + 1) * 128],
                                     dyn_in[:, c0:c1], start=False, stop=True)
                nc.vector.tensor_reduce(lnm[:, NB + b:NB + b + 1], vps[:],
                                        mybir.AxisListType.X, ALU.max)
                bv = wpool.tile([128, 1], F32, tag=f"bv{pi}")
                nc.vector.tensor_scalar_mul(bv[:], lnm[:, NB + b:NB + b + 1],
                                            nginveps)
                expo = epool.tile([128, S], F32, tag=f"expo{pi}")
                nc.scalar.activation(expo[:], vps[:], AF.Exp, bias=bv[:],
                                     scale=inveps, accum_out=sv[:, b:b + 1])
            nc.scalar.activation(lnm[:, 0:NB], sv[:], AF.Ln)
            nc.vector.tensor_add(lnm[:, NB:2 * NB], lnm[:, NB:2 * NB],
                                 halfn[:])
            if final:
                q = pi * 2 + (0 if fside else 1)
                dps = psq.tile([2 * NB, NB], F32, tag="dot")
                nc.tensor.matmul(dps[:], lnm[:], w[:])
                nc.scalar.copy(osum[0:2 * NB, q * NB:(q + 1) * NB], dps[:])
            else:
                tq = g["btab_view"][:, tq_off:tq_off + 1]
                qv = wpool.tile([128, NB], F32, tag=f"qv{pi}")
                nc.vector.tensor_scalar_mul(qv[:], lnm[:, NB:2 * NB], inveps)
                nc.vector.tensor_add(qv[:], qv[:], lnm[:, 0:NB])
                nc.vector.tensor_scalar_add(qv[:], qv[:], tq)
                qT = psq.tile([1, S], F32, tag="qT")
                for b in range(NB):
                    nc.tensor.matmul(qT[0:1, b * 128:(b + 1) * 128],
                                     qv[:, b:b + 1], g["ident"][:])
                nc.scalar.activation(
                    dyn_out[0:1, :], qT[:], AF.Copy, bias=0.0,
                    scale=g["rtab"][0:1, ne_off:ne_off + 1])

        g["btab_view"] = btab
        for t in range(NITER):
            for pr in probs:
                half(pr, t, True, False)
            for pr in probs:
                half(pr, t, False, False)
        for pr in probs:
            half(pr, NITER, True, True)
        for pr in probs:
            half(pr, NITER, False, True)

        nc.sync.dma_start(d_out[:], osum[0:16, :])
    nc.compile()
    return nc


# --------------------------------------------------------------------------
# cached-jit PJRT launcher (per-call jax.jit in run_bass_kernel_spmd costs
# ~0.3s of retracing; build the jitted callable once instead)
# --------------------------------------------------------------------------

def _make_runner(nc):
    bass2jax.install_neuronx_cc_hook()
    partition_name = (nc.partition_id_tensor.name
                      if nc.partition_id_tensor else None)
    in_names, out_names, out_avals, zero_shapes = [], [], [], []
    for alloc in nc.m.functions[0].allocations:
        if not isinstance(alloc, mybir.MemoryLocationSet):
            continue
        name = alloc.memorylocations[0].name
        if alloc.kind == "ExternalInput":
            if name != partition_name:
                in_names.append(name)
        elif alloc.kind == "ExternalOutput":
            shape = tuple(alloc.tensor_shape)
            dtype = mybir.dt.np(alloc.dtype)
            out_names.append(name)
            out_avals.append(jax.core.ShapedArray(shape, dtype))
            zero_shapes.append((shape, dtype))
    n_params = len(in_names)
    n_outs = len(out_avals)
    in_names_all = list(in_names) + list(out_names)
    if partition_name is not None:
        in_names_all.append(partition_name)
    donate = tuple(range(n_params, n_params + n_outs))

    def _body(*args):
        operands = list(args)
        if partition_name is not None:
            operands.append(bass2jax.partition_id_tensor())
        outs = bass2jax._bass_exec_p.bind(
            *operands, out_avals=tuple(out_avals),
            in_names=tuple(in_names_all), out_names=tuple(out_names),
            lowering_input_output_aliases=(), sim_require_finite=True,
            sim_require_nnan=True, nc=nc)
        return tuple(outs)

    devices = jax.devices()[:NCORES]
    mesh = Mesh(np.asarray(devices), ("core",))
    in_specs = (PartitionSpec("core"),) * (n_params + n_outs)
    out_specs = (PartitionSpec("core"),) * n_outs
    sharded = jax.jit(
        _shard_map(_body, mesh, in_specs, out_specs, False),
        donate_argnums=donate, keep_unused=True)

    def run(in_maps):
        concat_in = [
            np.concatenate([np.asarray(in_maps[c][nm]) for c in range(NCORES)],
                           axis=0)
            for nm in in_names]
        concat_zeros = [np.zeros((NCORES * s[0], *s[1:]), dt)
                        for s, dt in zero_shapes]
        out_arrs = sharded(*concat_in, *concat_zeros)
        return [
            {nm: np.asarray(out_arrs[i]).reshape(NCORES, *out_avals[i].shape)[c]
             for i, nm in enumerate(out_names)}
            for c in range(NCORES)]

    return run


# --------------------------------------------------------------------------
# host orchestration
# --------------------------------------------------------------------------

def _pk(vec, nb):
    """[nb*128] -> [128, nb]; column b holds points b*128..b*128+127."""
    return np.ascontiguousarray(vec.reshape(nb, 128).T)


def kernel(x, target, cluster_centers, filling_target, prediction_target):
    f32 = np.float32
    x = np.asarray(x, f32)
    y = np.asarray(target, f32)
    cc = np.asarray(cluster_centers, f32)
    filling_target = np.asarray(filling_target, f32)
    pt = np.asarray(prediction_target)

    nx = (x * x).sum(-1).astype(f32)
    ny = (y * y).sum(-1).astype(f32)
    ncc = (cc * cc).sum(-1).astype(f32)
    d_x = (nx[:, None] + ncc[None, :] - 2.0 * (x @ cc.T)).astype(f32)
    pred_x = d_x.argmin(1)

    idx_x = [np.where(pred_x == k)[0] for k in range(K)]
    idx_y = [np.where(pt == k)[0] for k in range(K)]
    nk = [len(i) for i in idx_x]
    mk = [len(i) for i in idx_y]
    S = _ceil128(max(max(nk), max(mk)))
    NB = S // 128
    OC = 6 * NB + 1

    # eps0 upper bounds per cost kind (exact max of C is not worth a launch)
    mx = np.sqrt(nx.max())
    my = np.sqrt(ny.max())
    eps0 = {"xy": max(f32(0.5 * (mx + my) ** 2), EPS),
            "xx": max(f32(0.5 * (2 * mx) ** 2), EPS),
            "yy": max(f32(0.5 * (2 * my) ** 2), EPS)}

    key = (S, NITER)
    if key not in _cache:
        nc = _build(S)
        _cache[key] = (nc, _make_runner(nc))
    nc, runner = _cache[key]

    t_arr = np.arange(NITER, dtype=f32)
    eps_seq = {}
    for kind, e0 in eps0.items():
        s = np.maximum(e0 * SCAL2 ** t_arr, EPS).astype(f32)
        eps_seq[kind] = np.concatenate([s, [EPS]]).astype(f32)
    kinds = ("xy", "xx", "yy")

    import ml_dtypes
    bf16 = ml_dtypes.bfloat16
    cpts = np.ascontiguousarray((-2.0 * cc.T).astype(bf16))
    cext = np.zeros((2, 8), f32)
    cext[0] = ncc
    cext[1] = -2.0

    in_maps = []
    host_terms = np.zeros(NCORES, f32)   # sum_p coeff * (aw.halfnx + bw.halfny)
    valid = np.zeros((NCORES, 3), f32)
    coeffs = np.array([1.0, -0.5, -0.5], f32)

    for k in range(K):
        xk = x[idx_x[k]]
        yk = y[idx_y[k]]
        cx, cy = nk[k], mk[k]
        nxk = nx[idx_x[k]]
        nyk = ny[idx_y[k]]

        def pts_tile(pts):
            t = np.zeros((64, S), bf16)
            t[:, :pts.shape[0]] = pts.T.astype(bf16)
            return t

        def ext_tile(n2):
            t = np.zeros((2, S), f32)
            t[0] = 1.0                     # h-row coefficient (all points)
            t[1, :n2.shape[0]] = -0.5 * n2
            return t

        xpts = pts_tile(xk)
        ypts = pts_tile(yk)
        xext = ext_tile(nxk)
        yext = ext_tile(nyk)

        lwx = f32(np.log(np.float64(1.0 / cx))) if cx else f32(0.0)
        lwy = f32(np.log(np.float64(1.0 / cy))) if cy else f32(0.0)
        # logw of the A (x/rows) and B (y/cols) side per problem
        lwA = (lwx, lwx, lwy)
        lwB = (lwy, lwx, lwy)

        inith = np.full((4, S), -BIG, f32)
        inith[0, :cy] = eps_seq["xy"][0] * lwy - 0.5 * nyk
        inith[1, :cx] = eps_seq["xx"][0] * lwx - 0.5 * nxk
        inith[2, :cy] = eps_seq["yy"][0] * lwy - 0.5 * nyk
        inith[3, :] = 1.0          # the constant ones row of the mov tiles

        hx = np.full(S, BIG, f32)
        hx[:cx] = 0.5 * nxk
        hy = np.full(S, BIG, f32)
        hy[:cy] = 0.5 * nyk
        awv = np.zeros(S, f32)
        if cx:
            awv[:cx] = f32(1.0 / cx)
        bwv = np.zeros(S, f32)
        if cy:
            bwv[:cy] = f32(1.0 / cy)

        bscal = np.zeros((1, 12 * NSEQ), f32)
        rtab = np.zeros((1, 3 * NSEQ), f32)
        for p, kind in enumerate(kinds):
            es = eps_seq[kind]
            bscal[0, p * NSEQ:(p + 1) * NSEQ] = 1.0 / es
            bscal[0, (3 + p) * NSEQ:(4 + p) * NSEQ] = -1.0 / es
            # tq tables: q += tq before the -eps_t-scaled writeback, so that
            # h' = eps_cons*logw - eps_t*q. f-side: cons = eps_t; g-side:
            # cons = eps_{t+1} (the f-half of the next iteration).
            bscal[0, (6 + p) * NSEQ:(7 + p) * NSEQ] = -lwA[p]
            tqg = np.zeros(NSEQ, f32)
            tqg[:NITER] = -(es[1:] / es[:NITER]) * lwB[p]
            bscal[0, (9 + p) * NSEQ:(10 + p) * NSEQ] = tqg
            rtab[0, p * NSEQ:(p + 1) * NSEQ] = -es

        in_maps.append({
            "xpts": xpts, "ypts": ypts, "xext": xext, "yext": yext,
            "inith": inith,
            "halfnx": _pk(hx, NB), "halfny": _pk(hy, NB),
            "aw": _pk(awv, NB), "bw": _pk(bwv, NB),
            "bscal": bscal, "rtab": rtab, "cpts": cpts, "cext": cext,
        })
        vk = f32(1.0) if (cx > 0 and cy > 0) else f32(0.0)
        valid[k] = vk
        ha = f32((awv * hx).sum(dtype=np.float64)) if cx else f32(0.0)
        hb = f32((bwv * hy).sum(dtype=np.float64)) if cy else f32(0.0)
        # per problem p: f-side host const uses A weights, g-side B weights
        hostA = (ha, ha, hb)
        hostB = (hb, ha, hb)
        # g2 consumes the t=NITER-1 f-half's h-row, whose logw bias used
        # eps_{NITER-1} instead of EPS; the resulting potential is uniformly
        # shifted by -(eps_{NITER-1}-EPS)*logw_A — add the exact shift back.
        delta = [float(eps_seq[kinds[p]][NITER - 1] - EPS) * float(lwA[p])
                 for p in range(3)]
        host_terms[k] = vk * float(
            sum(coeffs[p] * (hostA[p] + hostB[p] + delta[p])
                for p in range(3)))

    results = runner(in_maps)

    loss_med = np.float64(0.0)
    fill = np.zeros(8, np.float64)
    for k in range(K):
        o = results[k]["osum"].astype(np.float64)
        fill += nk[k] * o[0:8, OC - 1]
        for p in range(3):
            s_p = 0.0
            for side in range(2):
                q = p * 2 + side
                blk = o[0:2 * NB, q * NB:(q + 1) * NB]
                dln = sum(blk[b, b] for b in range(NB))
                dmp = sum(blk[NB + b, b] for b in range(NB))
                s_p += -float(EPS) * dln - dmp
            loss_med += valid[k, p] * coeffs[p] * s_p
        loss_med += host_terms[k]

    filling_x = (fill / N).astype(f32)
    loss_fil = np.mean((filling_x - filling_target) ** 2, dtype=f32)
    return np.asarray(f32(loss_fil + f32(loss_med)))


# revision 5
# speedup vs baseline: 1.1854x; 1.0506x over previous
"""Trainium2 Bass kernel for nn_LossKMeansWasserstein — single-launch design.

Architecture (v2): wall-clock in this axon-tunneled environment is dominated
by per-launch overhead (~0.35s stock, ~0.08s with a cached-jit launcher) and
host->device transfer (~50MB/s), not device compute (<1ms). So:

  1. ONE device launch per call (no cost-max prepass): eps0 per cost kind is
     replaced by the upper bound 0.5*(max|x|+max|y|)^2 (sim: <2e-4 effect on
     the loss at NITER=22 vs the reference's exact-max schedule).
  2. Cluster k lives entirely on core k: its 3 Sinkhorn problems (xy, xx, yy)
     run interleaved for cross-problem engine pipelining.
  3. Uploads are compact (~0.5MB/core): augmented point tiles + tiny scalar
     tables. Big operand tiles (moving side with the dynamic h-row) are built
     on device; per-(t,problem) eps scalars are broadcast to 128 partitions
     with a single ones-matmul.
  4. The jitted PJRT launcher is built once and cached; per-call overhead is
     concat + dispatch only.

Math: log-domain Sinkhorn on tilde-potentials G~ = g - 0.5|y|^2. The PE
computes V_ij = h_j + x_i.y_j - 0.5|x_i|^2 in one matmul per 128-row block
(h rides row 64 of the moving tile, -0.5|x|^2 rides row 65 of the stationary
tile). Row-max on DVE, fused exp+row-sum on ACT, then q = lnS + m'/eps is
transposed via PE and written back (scaled by -eps, biased by eps*log w) as
the next half-update's h-row.
"""
import os
import sys
from contextlib import ExitStack

import numpy as np

sys.path.insert(0, "/opt/trn_rl_repo")

import concourse.bass as bass  # noqa: E402
import concourse.tile as tile  # noqa: E402
from concourse import bacc, mybir, bass2jax  # noqa: E402
from concourse.masks import make_identity  # noqa: E402


class _PinActTables:
    """Steer Bacc's activation-table placement to the one set that holds
    BOTH exp and ln ('natural_log_exp_and_others'): the greedy pass
    otherwise alternates exp-only/ln-only sets, inserting ~276 table
    reloads (~350us of ACT time). Only the placement pass sees the
    filtered view; emitted act_func_set_ids stay valid act_info indices,
    and the pinned set genuinely contains every function we use.
    """

    def __enter__(self):
        self._orig = bacc.get_activation_tables

        def filtered(arch):
            tabs = self._orig(arch)
            both = {mybir.ActivationFunctionType.Exp,
                    mybir.ActivationFunctionType.Ln}
            out = {}
            for name, funcs in tabs.items():
                if name != "natural_log_exp_and_others" and both & funcs:
                    funcs = funcs - both
                out[name] = funcs
            return out

        bacc.get_activation_tables = filtered
        return self

    def __exit__(self, *exc):
        bacc.get_activation_tables = self._orig
        return False

import jax  # noqa: E402
from jax.sharding import Mesh, PartitionSpec  # noqa: E402

from jax.experimental.shard_map import shard_map as _sm  # noqa: E402


def _shard_map(f, mesh, in_specs, out_specs, check_rep):
    return _sm(f, mesh=mesh, in_specs=in_specs, out_specs=out_specs,
               check_rep=check_rep)

F32 = mybir.dt.float32
BF16 = mybir.dt.bfloat16
AF = mybir.ActivationFunctionType
ALU = mybir.AluOpType

N, M, D, K = 3072, 3072, 64, 8
BLUR = 0.05
EPS = np.float32(BLUR ** 2)
SCAL2 = np.float32(0.8 ** 2)
NITER = int(os.environ.get("KM_NITER", "22"))
NSEQ = NITER + 1
BIG = np.float32(1e7)
NCORES = 8

_cache = {}


def _ceil128(v):
    return max(128, ((v + 127) // 128) * 128)


# --------------------------------------------------------------------------
# device kernel
# --------------------------------------------------------------------------

def _build(S):
    NB = S // 128
    nc = bacc.Bacc("TRN2", target_bir_lowering=False, debug=False,
                   num_devices=NCORES)

    d = {}
    for name, shape, dt in (
        ("xpts", [64, S], BF16), ("ypts", [64, S], BF16),
        ("xext", [2, S], F32), ("yext", [2, S], F32),
        ("inith", [4, S], F32),
        ("halfnx", [128, NB], F32), ("halfny", [128, NB], F32),
        ("aw", [128, NB], F32), ("bw", [128, NB], F32),
        ("bscal", [1, 12 * NSEQ], F32), ("rtab", [1, 3 * NSEQ], F32),
        ("cpts", [64, 8], BF16), ("cext", [2, 8], F32),
    ):
        d[name] = nc.dram_tensor(name, shape, dt, kind="ExternalInput").ap()
    OC = 6 * NB + 1
    d_out = nc.dram_tensor("osum", [16, OC], F32, kind="ExternalOutput").ap()

    with tile.TileContext(nc) as tc, ExitStack() as ctx:
        cpool = ctx.enter_context(tc.tile_pool(name="cpool", bufs=1))
        g = {}
        for nm in ("xpts", "ypts", "xext", "yext", "inith", "halfnx",
                   "halfny", "aw", "bw", "bscal", "rtab", "cpts", "cext"):
            t = cpool.tile(list(d[nm].shape), d[nm].tensor.dtype,
                           tag=f"in_{nm}")
            nc.sync.dma_start(t[:], d[nm][:])
            g[nm] = t
        ident = cpool.tile([128, 128], F32, tag="ident")
        make_identity(nc, ident[:])
        g["ident"] = ident

        osum = cpool.tile([128, OC], F32)
        nc.vector.memset(osum[:], 0.0)

        # ---- dynamic-row tiles: row 0 = h (rewritten each half), row 1 = 1 ----
        dyns = {}
        for nm, hrow in (("dyn_b_xy", 0), ("dyn_a_xy", None),
                         ("dyn_b_xx", 1), ("dyn_a_xx", None),
                         ("dyn_b_yy", 2), ("dyn_a_yy", None)):
            dt_ = cpool.tile([2, S], F32, tag=nm)
            nc.sync.dma_start(dt_[1:2, :], g["inith"][3:4, :])
            if hrow is None:
                nc.vector.memset(dt_[0:1, :], 0.0)
            else:
                nc.sync.dma_start(dt_[0:1, :], g["inith"][hrow:hrow + 1, :])
            dyns[nm] = dt_

        # ---- broadcast per-(problem,t) scalars to 128 partitions ----
        onesrow = cpool.tile([1, 128], F32)
        nc.vector.memset(onesrow[:], 1.0)
        btab = cpool.tile([128, 12 * NSEQ], F32)
        with tc.tile_pool(name="setup_ps", bufs=1, space="PSUM") as sps:
            bc = sps.tile([128, 12 * NSEQ], F32, tag="bc")
            nc.tensor.matmul(bc[:], onesrow[:], g["bscal"][:])
            nc.scalar.copy(btab[:], bc[:])

            # ---- filling partial sums (independent of sinkhorn) ----
            fillps = sps.tile([8, 1], F32, tag="fillps")
            for b in range(NB):
                dxp = sps.tile([128, 8], F32, tag="dxp")
                nc.tensor.matmul(dxp[:], g["xpts"][:, b * 128:(b + 1) * 128],
                                 g["cpts"][:], start=True, stop=False)
                nc.tensor.matmul(dxp[:], g["xext"][:, b * 128:(b + 1) * 128],
                                 g["cext"][:], start=False, stop=True)
                mind = cpool.tile([128, 1], F32, tag="mind")
                nc.vector.tensor_reduce(mind[:], dxp[:], mybir.AxisListType.X,
                                        ALU.min)
                et = cpool.tile([128, 8], F32, tag="et")
                ssum = cpool.tile([128, 1], F32, tag="ssum")
                nc.scalar.activation(et[:], dxp[:], AF.Exp, bias=mind[:],
                                     scale=-1.0, accum_out=ssum[:])
                rs = cpool.tile([128, 1], F32, tag="rs")
                nc.vector.reciprocal(rs[:], ssum[:])
                soft = cpool.tile([128, 8], F32, tag="soft")
                nc.vector.tensor_scalar_mul(soft[:], et[:], rs[:])
                nc.tensor.matmul(fillps[:], soft[:], g["aw"][:, b:b + 1],
                                 start=(b == 0), stop=(b == NB - 1))
            nc.scalar.copy(osum[0:8, OC - 1:OC], fillps[:])

        # ---- the 3 sinkhorn problems, interleaved ----
        probs = [
            dict(pi=0, ptsA=g["xpts"], ptsB=g["ypts"],
                 extA=g["xext"], extB=g["yext"],
                 dynA=dyns["dyn_a_xy"], dynB=dyns["dyn_b_xy"],
                 hA=g["halfnx"], hB=g["halfny"], wA=g["aw"], wB=g["bw"]),
            dict(pi=1, ptsA=g["xpts"], ptsB=g["xpts"],
                 extA=g["xext"], extB=g["xext"],
                 dynA=dyns["dyn_a_xx"], dynB=dyns["dyn_b_xx"],
                 hA=g["halfnx"], hB=g["halfnx"], wA=g["aw"], wB=g["aw"]),
            dict(pi=2, ptsA=g["ypts"], ptsB=g["ypts"],
                 extA=g["yext"], extB=g["yext"],
                 dynA=dyns["dyn_a_yy"], dynB=dyns["dyn_b_yy"],
                 hA=g["halfny"], hB=g["halfny"], wA=g["bw"], wB=g["bw"]),
        ]

        psv = ctx.enter_context(tc.tile_pool(name="psv", bufs=2, space="PSUM"))
        psq = ctx.enter_context(tc.tile_pool(name="psq", bufs=1, space="PSUM"))
        wpool = ctx.enter_context(tc.tile_pool(name="wpool", bufs=2))
        epool = ctx.enter_context(tc.tile_pool(name="epool", bufs=1))

        def half(pr, t, fside, final):
            pi = pr["pi"]
            if fside:
                ptsS, ptsM = pr["ptsA"], pr["ptsB"]
                extS, dyn_in, dyn_out = pr["extA"], pr["dynB"], pr["dynA"]
                halfn, w = pr["hA"], pr["wA"]
                # h' consumed by the g-half of the SAME iteration t
                tq_off = (6 + pi) * NSEQ + t        # -logw_A
            else:
                ptsS, ptsM = pr["ptsB"], pr["ptsA"]
                extS, dyn_in, dyn_out = pr["extB"], pr["dynA"], pr["dynB"]
                halfn, w = pr["hB"], pr["wB"]
                # h' consumed by the f-half of iteration t+1 (incl. final)
                tq_off = (9 + pi) * NSEQ + t        # -(eps_{t+1}/eps_t)*logw_B
            inveps = g["btab_view"][:, pi * NSEQ + t:pi * NSEQ + t + 1]
            nginveps = g["btab_view"][:, (3 + pi) * NSEQ + t:
                                      (3 + pi) * NSEQ + t + 1]
            ne_off = pi * NSEQ + t                  # -eps_t

            sd = "f" if fside else "g"
            lnm = wpool.tile([128, 2 * NB], F32, tag=f"lnm{pi}{sd}")
            sv = wpool.tile([128, NB], F32, tag=f"sv{pi}{sd}")
            for b in range(NB):
                vps = psv.tile([128, S], F32, tag="vps")
                for c0 in range(0, S, 512):
                    c1 = min(c0 + 512, S)
                    nc.tensor.matmul(vps[:, c0:c1],
                                     ptsS[:, b * 128:(b + 1) * 128],
                                     ptsM[:, c0:c1], start=True, stop=False)
                    nc.tensor.matmul(vps[:, c0:c1],
                                     extS[:, b * 128:(b + 1) * 128],
                                     dyn_in[:, c0:c1], start=False, stop=True)
                nc.vector.tensor_reduce(lnm[:, NB + b:NB + b + 1], vps[:],
                                        mybir.AxisListType.X, ALU.max)
                bv = wpool.tile([128, 1], F32, tag=f"bv{pi}")
                nc.vector.tensor_scalar_mul(bv[:], lnm[:, NB + b:NB + b + 1],
                                            nginveps)
                expo = epool.tile([128, S], F32, tag=f"expo{pi}")
                nc.scalar.activation(expo[:], vps[:], AF.Exp, bias=bv[:],
                                     scale=inveps, accum_out=sv[:, b:b + 1])
            nc.scalar.activation(lnm[:, 0:NB], sv[:], AF.Ln)
            nc.vector.tensor_add(lnm[:, NB:2 * NB], lnm[:, NB:2 * NB],
                                 halfn[:])
            if final:
                q = pi * 2 + (0 if fside else 1)
                dps = psq.tile([2 * NB, NB], F32, tag="dot")
                nc.tensor.matmul(dps[:], lnm[:], w[:])
                nc.scalar.copy(osum[0:2 * NB, q * NB:(q + 1) * NB], dps[:])
            else:
                tq = g["btab_view"][:, tq_off:tq_off + 1]
                qv = wpool.tile([128, NB], F32, tag=f"qv{pi}")
                nc.vector.tensor_scalar_mul(qv[:], lnm[:, NB:2 * NB], inveps)
                nc.vector.tensor_add(qv[:], qv[:], lnm[:, 0:NB])
                nc.vector.tensor_scalar_add(qv[:], qv[:], tq)
                qT = psq.tile([1, S], F32, tag="qT")
                for b in range(NB):
                    nc.tensor.matmul(qT[0:1, b * 128:(b + 1) * 128],
                                     qv[:, b:b + 1], g["ident"][:])
                nc.scalar.activation(
                    dyn_out[0:1, :], qT[:], AF.Copy, bias=0.0,
                    scale=g["rtab"][0:1, ne_off:ne_off + 1])

        g["btab_view"] = btab
        for t in range(NITER):
            for pr in probs:
                half(pr, t, True, False)
            for pr in probs:
                half(pr, t, False, False)
        for pr in probs:
            half(pr, NITER, True, True)
        for pr in probs:
            half(pr, NITER, False, True)

        nc.sync.dma_start(d_out[:], osum[0:16, :])
    with _PinActTables():
        nc.compile()
    return nc


# --------------------------------------------------------------------------
# cached-jit PJRT launcher (per-call jax.jit in run_bass_kernel_spmd costs
# ~0.3s of retracing; build the jitted callable once instead)
# --------------------------------------------------------------------------

def _make_runner(nc):
    bass2jax.install_neuronx_cc_hook()
    partition_name = (nc.partition_id_tensor.name
                      if nc.partition_id_tensor else None)
    in_names, out_names, out_avals, zero_shapes = [], [], [], []
    for alloc in nc.m.functions[0].allocations:
        if not isinstance(alloc, mybir.MemoryLocationSet):
            continue
        name = alloc.memorylocations[0].name
        if alloc.kind == "ExternalInput":
            if name != partition_name:
                in_names.append(name)
        elif alloc.kind == "ExternalOutput":
            shape = tuple(alloc.tensor_shape)
            dtype = mybir.dt.np(alloc.dtype)
            out_names.append(name)
            out_avals.append(jax.core.ShapedArray(shape, dtype))
            zero_shapes.append((shape, dtype))
    n_params = len(in_names)
    n_outs = len(out_avals)
    in_names_all = list(in_names) + list(out_names)
    if partition_name is not None:
        in_names_all.append(partition_name)
    donate = tuple(range(n_params, n_params + n_outs))

    def _body(*args):
        operands = list(args)
        if partition_name is not None:
            operands.append(bass2jax.partition_id_tensor())
        outs = bass2jax._bass_exec_p.bind(
            *operands, out_avals=tuple(out_avals),
            in_names=tuple(in_names_all), out_names=tuple(out_names),
            lowering_input_output_aliases=(), sim_require_finite=True,
            sim_require_nnan=True, nc=nc)
        return tuple(outs)

    devices = jax.devices()[:NCORES]
    mesh = Mesh(np.asarray(devices), ("core",))
    in_specs = (PartitionSpec("core"),) * (n_params + n_outs)
    out_specs = (PartitionSpec("core"),) * n_outs
    sharded = jax.jit(
        _shard_map(_body, mesh, in_specs, out_specs, False),
        donate_argnums=donate, keep_unused=True)

    def run(in_maps):
        concat_in = [
            np.concatenate([np.asarray(in_maps[c][nm]) for c in range(NCORES)],
                           axis=0)
            for nm in in_names]
        concat_zeros = [np.zeros((NCORES * s[0], *s[1:]), dt)
                        for s, dt in zero_shapes]
        out_arrs = sharded(*concat_in, *concat_zeros)
        return [
            {nm: np.asarray(out_arrs[i]).reshape(NCORES, *out_avals[i].shape)[c]
             for i, nm in enumerate(out_names)}
            for c in range(NCORES)]

    return run


# --------------------------------------------------------------------------
# host orchestration
# --------------------------------------------------------------------------

def _pk(vec, nb):
    """[nb*128] -> [128, nb]; column b holds points b*128..b*128+127."""
    return np.ascontiguousarray(vec.reshape(nb, 128).T)


def kernel(x, target, cluster_centers, filling_target, prediction_target):
    f32 = np.float32
    x = np.asarray(x, f32)
    y = np.asarray(target, f32)
    cc = np.asarray(cluster_centers, f32)
    filling_target = np.asarray(filling_target, f32)
    pt = np.asarray(prediction_target)

    nx = (x * x).sum(-1).astype(f32)
    ny = (y * y).sum(-1).astype(f32)
    ncc = (cc * cc).sum(-1).astype(f32)
    d_x = (nx[:, None] + ncc[None, :] - 2.0 * (x @ cc.T)).astype(f32)
    pred_x = d_x.argmin(1)

    idx_x = [np.where(pred_x == k)[0] for k in range(K)]
    idx_y = [np.where(pt == k)[0] for k in range(K)]
    nk = [len(i) for i in idx_x]
    mk = [len(i) for i in idx_y]
    S = _ceil128(max(max(nk), max(mk)))
    NB = S // 128
    OC = 6 * NB + 1

    # eps0 upper bounds per cost kind (exact max of C is not worth a launch)
    mx = np.sqrt(nx.max())
    my = np.sqrt(ny.max())
    eps0 = {"xy": max(f32(0.5 * (mx + my) ** 2), EPS),
            "xx": max(f32(0.5 * (2 * mx) ** 2), EPS),
            "yy": max(f32(0.5 * (2 * my) ** 2), EPS)}

    key = (S, NITER)
    if key not in _cache:
        nc = _build(S)
        _cache[key] = (nc, _make_runner(nc))
    nc, runner = _cache[key]

    t_arr = np.arange(NITER, dtype=f32)
    eps_seq = {}
    for kind, e0 in eps0.items():
        s = np.maximum(e0 * SCAL2 ** t_arr, EPS).astype(f32)
        eps_seq[kind] = np.concatenate([s, [EPS]]).astype(f32)
    kinds = ("xy", "xx", "yy")

    import ml_dtypes
    bf16 = ml_dtypes.bfloat16
    cpts = np.ascontiguousarray((-2.0 * cc.T).astype(bf16))
    cext = np.zeros((2, 8), f32)
    cext[0] = ncc
    cext[1] = -2.0

    in_maps = []
    host_terms = np.zeros(NCORES, f32)   # sum_p coeff * (aw.halfnx + bw.halfny)
    valid = np.zeros((NCORES, 3), f32)
    coeffs = np.array([1.0, -0.5, -0.5], f32)

    for k in range(K):
        xk = x[idx_x[k]]
        yk = y[idx_y[k]]
        cx, cy = nk[k], mk[k]
        nxk = nx[idx_x[k]]
        nyk = ny[idx_y[k]]

        def pts_tile(pts):
            t = np.zeros((64, S), bf16)
            t[:, :pts.shape[0]] = pts.T.astype(bf16)
            return t

        def ext_tile(n2):
            t = np.zeros((2, S), f32)
            t[0] = 1.0                     # h-row coefficient (all points)
            t[1, :n2.shape[0]] = -0.5 * n2
            return t

        xpts = pts_tile(xk)
        ypts = pts_tile(yk)
        xext = ext_tile(nxk)
        yext = ext_tile(nyk)

        lwx = f32(np.log(np.float64(1.0 / cx))) if cx else f32(0.0)
        lwy = f32(np.log(np.float64(1.0 / cy))) if cy else f32(0.0)
        # logw of the A (x/rows) and B (y/cols) side per problem
        lwA = (lwx, lwx, lwy)
        lwB = (lwy, lwx, lwy)

        inith = np.full((4, S), -BIG, f32)
        inith[0, :cy] = eps_seq["xy"][0] * lwy - 0.5 * nyk
        inith[1, :cx] = eps_seq["xx"][0] * lwx - 0.5 * nxk
        inith[2, :cy] = eps_seq["yy"][0] * lwy - 0.5 * nyk
        inith[3, :] = 1.0          # the constant ones row of the mov tiles

        hx = np.full(S, BIG, f32)
        hx[:cx] = 0.5 * nxk
        hy = np.full(S, BIG, f32)
        hy[:cy] = 0.5 * nyk
        awv = np.zeros(S, f32)
        if cx:
            awv[:cx] = f32(1.0 / cx)
        bwv = np.zeros(S, f32)
        if cy:
            bwv[:cy] = f32(1.0 / cy)

        bscal = np.zeros((1, 12 * NSEQ), f32)
        rtab = np.zeros((1, 3 * NSEQ), f32)
        for p, kind in enumerate(kinds):
            es = eps_seq[kind]
            bscal[0, p * NSEQ:(p + 1) * NSEQ] = 1.0 / es
            bscal[0, (3 + p) * NSEQ:(4 + p) * NSEQ] = -1.0 / es
            # tq tables: q += tq before the -eps_t-scaled writeback, so that
            # h' = eps_cons*logw - eps_t*q. f-side: cons = eps_t; g-side:
            # cons = eps_{t+1} (the f-half of the next iteration).
            bscal[0, (6 + p) * NSEQ:(7 + p) * NSEQ] = -lwA[p]
            tqg = np.zeros(NSEQ, f32)
            tqg[:NITER] = -(es[1:] / es[:NITER]) * lwB[p]
            bscal[0, (9 + p) * NSEQ:(10 + p) * NSEQ] = tqg
            rtab[0, p * NSEQ:(p + 1) * NSEQ] = -es

        in_maps.append({
            "xpts": xpts, "ypts": ypts, "xext": xext, "yext": yext,
            "inith": inith,
            "halfnx": _pk(hx, NB), "halfny": _pk(hy, NB),
            "aw": _pk(awv, NB), "bw": _pk(bwv, NB),
            "bscal": bscal, "rtab": rtab, "cpts": cpts, "cext": cext,
        })
        vk = f32(1.0) if (cx > 0 and cy > 0) else f32(0.0)
        valid[k] = vk
        ha = f32((awv * hx).sum(dtype=np.float64)) if cx else f32(0.0)
        hb = f32((bwv * hy).sum(dtype=np.float64)) if cy else f32(0.0)
        # per problem p: f-side host const uses A weights, g-side B weights
        hostA = (ha, ha, hb)
        hostB = (hb, ha, hb)
        # g2 consumes the t=NITER-1 f-half's h-row, whose logw bias used
        # eps_{NITER-1} instead of EPS; the resulting potential is uniformly
        # shifted by -(eps_{NITER-1}-EPS)*logw_A — add the exact shift back.
        delta = [float(eps_seq[kinds[p]][NITER - 1] - EPS) * float(lwA[p])
                 for p in range(3)]
        host_terms[k] = vk * float(
            sum(coeffs[p] * (hostA[p] + hostB[p] + delta[p])
                for p in range(3)))

    results = runner(in_maps)

    loss_med = np.float64(0.0)
    fill = np.zeros(8, np.float64)
    for k in range(K):
        o = results[k]["osum"].astype(np.float64)
        fill += nk[k] * o[0:8, OC - 1]
        for p in range(3):
            s_p = 0.0
            for side in range(2):
                q = p * 2 + side
                blk = o[0:2 * NB, q * NB:(q + 1) * NB]
                dln = sum(blk[b, b] for b in range(NB))
                dmp = sum(blk[NB + b, b] for b in range(NB))
                s_p += -float(EPS) * dln - dmp
            loss_med += valid[k, p] * coeffs[p] * s_p
        loss_med += host_terms[k]

    filling_x = (fill / N).astype(f32)
    loss_fil = np.mean((filling_x - filling_target) ** 2, dtype=f32)
    return np.asarray(f32(loss_fil + f32(loss_med)))


# revision 6
# speedup vs baseline: 1.2137x; 1.0239x over previous
"""Trainium2 Bass kernel for nn_LossKMeansWasserstein — single-launch design.

Architecture (v2): wall-clock in this axon-tunneled environment is dominated
by per-launch overhead (~0.35s stock, ~0.08s with a cached-jit launcher) and
host->device transfer (~50MB/s), not device compute (<1ms). So:

  1. ONE device launch per call (no cost-max prepass): eps0 per cost kind is
     replaced by the upper bound 0.5*(max|x|+max|y|)^2 (sim: <2e-4 effect on
     the loss at NITER=22 vs the reference's exact-max schedule).
  2. Cluster k lives entirely on core k: its 3 Sinkhorn problems (xy, xx, yy)
     run interleaved for cross-problem engine pipelining.
  3. Uploads are compact (~0.5MB/core): augmented point tiles + tiny scalar
     tables. Big operand tiles (moving side with the dynamic h-row) are built
     on device; per-(t,problem) eps scalars are broadcast to 128 partitions
     with a single ones-matmul.
  4. The jitted PJRT launcher is built once and cached; per-call overhead is
     concat + dispatch only.

Math: log-domain Sinkhorn on tilde-potentials G~ = g - 0.5|y|^2. The PE
computes V_ij = h_j + x_i.y_j - 0.5|x_i|^2 in one matmul per 128-row block
(h rides row 64 of the moving tile, -0.5|x|^2 rides row 65 of the stationary
tile). Row-max on DVE, fused exp+row-sum on ACT, then q = lnS + m'/eps is
transposed via PE and written back (scaled by -eps, biased by eps*log w) as
the next half-update's h-row.
"""
import os
import sys
from contextlib import ExitStack

import numpy as np

sys.path.insert(0, "/opt/trn_rl_repo")

import concourse.bass as bass  # noqa: E402
import concourse.tile as tile  # noqa: E402
from concourse import bacc, mybir, bass2jax  # noqa: E402
from concourse.masks import make_identity  # noqa: E402


class _PinActTables:
    """Steer Bacc's activation-table placement to the one set that holds
    BOTH exp and ln ('natural_log_exp_and_others'): the greedy pass
    otherwise alternates exp-only/ln-only sets, inserting ~276 table
    reloads (~350us of ACT time). Only the placement pass sees the
    filtered view; emitted act_func_set_ids stay valid act_info indices,
    and the pinned set genuinely contains every function we use.
    """

    def __enter__(self):
        self._orig = bacc.get_activation_tables

        def filtered(arch):
            tabs = self._orig(arch)
            both = {mybir.ActivationFunctionType.Exp,
                    mybir.ActivationFunctionType.Ln}
            combined = "natural_log_exp_and_others"
            if not both <= tabs.get(combined, set()):
                return tabs            # unexpected act_info: leave untouched
            out = {}
            for name, funcs in tabs.items():
                if name != combined and both & funcs:
                    funcs = funcs - both
                out[name] = funcs
            return out

        bacc.get_activation_tables = filtered
        return self

    def __exit__(self, *exc):
        bacc.get_activation_tables = self._orig
        return False

import jax  # noqa: E402
from jax.sharding import Mesh, PartitionSpec  # noqa: E402

from jax.experimental.shard_map import shard_map as _sm  # noqa: E402


def _shard_map(f, mesh, in_specs, out_specs, check_rep):
    return _sm(f, mesh=mesh, in_specs=in_specs, out_specs=out_specs,
               check_rep=check_rep)

F32 = mybir.dt.float32
BF16 = mybir.dt.bfloat16
AF = mybir.ActivationFunctionType
ALU = mybir.AluOpType

N, M, D, K = 3072, 3072, 64, 8
BLUR = 0.05
EPS = np.float32(BLUR ** 2)
SCAL2 = np.float32(0.8 ** 2)
NITER = int(os.environ.get("KM_NITER", "22"))
NSEQ = NITER + 1
BIG = np.float32(1e7)
NCORES = 8

_cache = {}


def _ceil128(v):
    return max(128, ((v + 127) // 128) * 128)


# --------------------------------------------------------------------------
# device kernel
# --------------------------------------------------------------------------

def _build(S):
    NB = S // 128
    nc = bacc.Bacc("TRN2", target_bir_lowering=False, debug=False,
                   num_devices=NCORES)

    d = {}
    for name, shape, dt in (
        ("xpts", [64, S], BF16), ("ypts", [64, S], BF16),
        ("xext", [2, S], F32), ("yext", [2, S], F32),
        ("inith", [4, S], F32),
        ("halfnx", [128, NB], F32), ("halfny", [128, NB], F32),
        ("aw", [128, NB], F32), ("bw", [128, NB], F32),
        ("bscal", [1, 12 * NSEQ], F32), ("rtab", [1, 3 * NSEQ], F32),
        ("cpts", [64, 8], BF16), ("cext", [2, 8], F32),
    ):
        d[name] = nc.dram_tensor(name, shape, dt, kind="ExternalInput").ap()
    OC = 6 * NB + 1
    d_out = nc.dram_tensor("osum", [16, OC], F32, kind="ExternalOutput").ap()

    with tile.TileContext(nc) as tc, ExitStack() as ctx:
        cpool = ctx.enter_context(tc.tile_pool(name="cpool", bufs=1))
        g = {}
        for nm in ("xpts", "ypts", "xext", "yext", "inith", "halfnx",
                   "halfny", "aw", "bw", "bscal", "rtab", "cpts", "cext"):
            t = cpool.tile(list(d[nm].shape), d[nm].tensor.dtype,
                           tag=f"in_{nm}")
            nc.sync.dma_start(t[:], d[nm][:])
            g[nm] = t
        ident = cpool.tile([128, 128], F32, tag="ident")
        make_identity(nc, ident[:])
        g["ident"] = ident

        osum = cpool.tile([128, OC], F32)
        nc.vector.memset(osum[:], 0.0)

        # ---- dynamic-row tiles: row 0 = h (rewritten each half), row 1 = 1 ----
        dyns = {}
        for nm, hrow in (("dyn_b_xy", 0), ("dyn_a_xy", None),
                         ("dyn_b_xx", 1), ("dyn_a_xx", None),
                         ("dyn_b_yy", 2), ("dyn_a_yy", None)):
            dt_ = cpool.tile([2, S], F32, tag=nm)
            nc.sync.dma_start(dt_[1:2, :], g["inith"][3:4, :])
            if hrow is None:
                nc.vector.memset(dt_[0:1, :], 0.0)
            else:
                nc.sync.dma_start(dt_[0:1, :], g["inith"][hrow:hrow + 1, :])
            dyns[nm] = dt_

        # ---- broadcast per-(problem,t) scalars to 128 partitions ----
        onesrow = cpool.tile([1, 128], F32)
        nc.vector.memset(onesrow[:], 1.0)
        btab = cpool.tile([128, 12 * NSEQ], F32)
        with tc.tile_pool(name="setup_ps", bufs=1, space="PSUM") as sps:
            bc = sps.tile([128, 12 * NSEQ], F32, tag="bc")
            nc.tensor.matmul(bc[:], onesrow[:], g["bscal"][:])
            nc.scalar.copy(btab[:], bc[:])

            # ---- filling partial sums (independent of sinkhorn) ----
            fillps = sps.tile([8, 1], F32, tag="fillps")
            for b in range(NB):
                dxp = sps.tile([128, 8], F32, tag="dxp")
                nc.tensor.matmul(dxp[:], g["xpts"][:, b * 128:(b + 1) * 128],
                                 g["cpts"][:], start=True, stop=False)
                nc.tensor.matmul(dxp[:], g["xext"][:, b * 128:(b + 1) * 128],
                                 g["cext"][:], start=False, stop=True)
                mind = cpool.tile([128, 1], F32, tag="mind")
                nc.vector.tensor_reduce(mind[:], dxp[:], mybir.AxisListType.X,
                                        ALU.min)
                et = cpool.tile([128, 8], F32, tag="et")
                ssum = cpool.tile([128, 1], F32, tag="ssum")
                nc.scalar.activation(et[:], dxp[:], AF.Exp, bias=mind[:],
                                     scale=-1.0, accum_out=ssum[:])
                rs = cpool.tile([128, 1], F32, tag="rs")
                nc.vector.reciprocal(rs[:], ssum[:])
                soft = cpool.tile([128, 8], F32, tag="soft")
                nc.vector.tensor_scalar_mul(soft[:], et[:], rs[:])
                nc.tensor.matmul(fillps[:], soft[:], g["aw"][:, b:b + 1],
                                 start=(b == 0), stop=(b == NB - 1))
            nc.scalar.copy(osum[0:8, OC - 1:OC], fillps[:])

        # ---- the 3 sinkhorn problems, interleaved ----
        probs = [
            dict(pi=0, ptsA=g["xpts"], ptsB=g["ypts"],
                 extA=g["xext"], extB=g["yext"],
                 dynA=dyns["dyn_a_xy"], dynB=dyns["dyn_b_xy"],
                 hA=g["halfnx"], hB=g["halfny"], wA=g["aw"], wB=g["bw"]),
            dict(pi=1, ptsA=g["xpts"], ptsB=g["xpts"],
                 extA=g["xext"], extB=g["xext"],
                 dynA=dyns["dyn_a_xx"], dynB=dyns["dyn_b_xx"],
                 hA=g["halfnx"], hB=g["halfnx"], wA=g["aw"], wB=g["aw"]),
            dict(pi=2, ptsA=g["ypts"], ptsB=g["ypts"],
                 extA=g["yext"], extB=g["yext"],
                 dynA=dyns["dyn_a_yy"], dynB=dyns["dyn_b_yy"],
                 hA=g["halfny"], hB=g["halfny"], wA=g["bw"], wB=g["bw"]),
        ]

        psv = ctx.enter_context(tc.tile_pool(name="psv", bufs=2, space="PSUM"))
        psq = ctx.enter_context(tc.tile_pool(name="psq", bufs=1, space="PSUM"))
        wpool = ctx.enter_context(tc.tile_pool(name="wpool", bufs=2))
        epool = ctx.enter_context(tc.tile_pool(name="epool", bufs=1))

        def half(pr, t, fside, final):
            pi = pr["pi"]
            if fside:
                ptsS, ptsM = pr["ptsA"], pr["ptsB"]
                extS, dyn_in, dyn_out = pr["extA"], pr["dynB"], pr["dynA"]
                halfn, w = pr["hA"], pr["wA"]
                # h' consumed by the g-half of the SAME iteration t
                tq_off = (6 + pi) * NSEQ + t        # -logw_A
            else:
                ptsS, ptsM = pr["ptsB"], pr["ptsA"]
                extS, dyn_in, dyn_out = pr["extB"], pr["dynA"], pr["dynB"]
                halfn, w = pr["hB"], pr["wB"]
                # h' consumed by the f-half of iteration t+1 (incl. final)
                tq_off = (9 + pi) * NSEQ + t        # -(eps_{t+1}/eps_t)*logw_B
            inveps = g["btab_view"][:, pi * NSEQ + t:pi * NSEQ + t + 1]
            nginveps = g["btab_view"][:, (3 + pi) * NSEQ + t:
                                      (3 + pi) * NSEQ + t + 1]
            ne_off = pi * NSEQ + t                  # -eps_t

            sd = "f" if fside else "g"
            lnm = wpool.tile([128, 2 * NB], F32, tag=f"lnm{pi}{sd}")
            sv = wpool.tile([128, NB], F32, tag=f"sv{pi}{sd}")
            for b in range(NB):
                vps = psv.tile([128, S], F32, tag="vps")
                for c0 in range(0, S, 512):
                    c1 = min(c0 + 512, S)
                    nc.tensor.matmul(vps[:, c0:c1],
                                     ptsS[:, b * 128:(b + 1) * 128],
                                     ptsM[:, c0:c1], start=True, stop=False)
                    nc.tensor.matmul(vps[:, c0:c1],
                                     extS[:, b * 128:(b + 1) * 128],
                                     dyn_in[:, c0:c1], start=False, stop=True)
                nc.vector.tensor_reduce(lnm[:, NB + b:NB + b + 1], vps[:],
                                        mybir.AxisListType.X, ALU.max)
                bv = wpool.tile([128, 1], F32, tag=f"bv{pi}")
                nc.vector.tensor_scalar_mul(bv[:], lnm[:, NB + b:NB + b + 1],
                                            nginveps)
                expo = epool.tile([128, S], F32, tag=f"expo{pi}")
                nc.scalar.activation(expo[:], vps[:], AF.Exp, bias=bv[:],
                                     scale=inveps, accum_out=sv[:, b:b + 1])
            nc.scalar.activation(lnm[:, 0:NB], sv[:], AF.Ln)
            nc.vector.tensor_add(lnm[:, NB:2 * NB], lnm[:, NB:2 * NB],
                                 halfn[:])
            if final:
                q = pi * 2 + (0 if fside else 1)
                dps = psq.tile([2 * NB, NB], F32, tag="dot")
                nc.tensor.matmul(dps[:], lnm[:], w[:])
                nc.scalar.copy(osum[0:2 * NB, q * NB:(q + 1) * NB], dps[:])
            else:
                tq = g["btab_view"][:, tq_off:tq_off + 1]
                qv = wpool.tile([128, NB], F32, tag=f"qv{pi}")
                nc.vector.tensor_scalar_mul(qv[:], lnm[:, NB:2 * NB], inveps)
                nc.vector.tensor_add(qv[:], qv[:], lnm[:, 0:NB])
                nc.vector.tensor_scalar_add(qv[:], qv[:], tq)
                qT = psq.tile([1, S], F32, tag="qT")
                for b in range(NB):
                    nc.tensor.matmul(qT[0:1, b * 128:(b + 1) * 128],
                                     qv[:, b:b + 1], g["ident"][:])
                nc.scalar.activation(
                    dyn_out[0:1, :], qT[:], AF.Copy, bias=0.0,
                    scale=g["rtab"][0:1, ne_off:ne_off + 1])

        g["btab_view"] = btab
        for t in range(NITER):
            for pr in probs:
                half(pr, t, True, False)
            for pr in probs:
                half(pr, t, False, False)
        for pr in probs:
            half(pr, NITER, True, True)
        for pr in probs:
            half(pr, NITER, False, True)

        nc.sync.dma_start(d_out[:], osum[0:16, :])
    with _PinActTables():
        nc.compile()
    return nc


# --------------------------------------------------------------------------
# cached-jit PJRT launcher (per-call jax.jit in run_bass_kernel_spmd costs
# ~0.3s of retracing; build the jitted callable once instead)
# --------------------------------------------------------------------------

def _make_runner(nc):
    bass2jax.install_neuronx_cc_hook()
    partition_name = (nc.partition_id_tensor.name
                      if nc.partition_id_tensor else None)
    in_names, out_names, out_avals, zero_shapes = [], [], [], []
    for alloc in nc.m.functions[0].allocations:
        if not isinstance(alloc, mybir.MemoryLocationSet):
            continue
        name = alloc.memorylocations[0].name
        if alloc.kind == "ExternalInput":
            if name != partition_name:
                in_names.append(name)
        elif alloc.kind == "ExternalOutput":
            shape = tuple(alloc.tensor_shape)
            dtype = mybir.dt.np(alloc.dtype)
            out_names.append(name)
            out_avals.append(jax.core.ShapedArray(shape, dtype))
            zero_shapes.append((shape, dtype))
    n_params = len(in_names)
    n_outs = len(out_avals)
    in_names_all = list(in_names) + list(out_names)
    if partition_name is not None:
        in_names_all.append(partition_name)
    donate = tuple(range(n_params, n_params + n_outs))

    def _body(*args):
        operands = list(args)
        if partition_name is not None:
            operands.append(bass2jax.partition_id_tensor())
        outs = bass2jax._bass_exec_p.bind(
            *operands, out_avals=tuple(out_avals),
            in_names=tuple(in_names_all), out_names=tuple(out_names),
            lowering_input_output_aliases=(), sim_require_finite=True,
            sim_require_nnan=True, nc=nc)
        return tuple(outs)

    devices = jax.devices()[:NCORES]
    mesh = Mesh(np.asarray(devices), ("core",))
    in_specs = (PartitionSpec("core"),) * (n_params + n_outs)
    out_specs = (PartitionSpec("core"),) * n_outs
    sharded = jax.jit(
        _shard_map(_body, mesh, in_specs, out_specs, False),
        donate_argnums=donate, keep_unused=True)

    def run(in_maps):
        concat_in = [
            np.concatenate([np.asarray(in_maps[c][nm]) for c in range(NCORES)],
                           axis=0)
            for nm in in_names]
        concat_zeros = [np.zeros((NCORES * s[0], *s[1:]), dt)
                        for s, dt in zero_shapes]
        out_arrs = sharded(*concat_in, *concat_zeros)
        return [
            {nm: np.asarray(out_arrs[i]).reshape(NCORES, *out_avals[i].shape)[c]
             for i, nm in enumerate(out_names)}
            for c in range(NCORES)]

    return run


# --------------------------------------------------------------------------
# host orchestration
# --------------------------------------------------------------------------

def _pk(vec, nb):
    """[nb*128] -> [128, nb]; column b holds points b*128..b*128+127."""
    return np.ascontiguousarray(vec.reshape(nb, 128).T)


def kernel(x, target, cluster_centers, filling_target, prediction_target):
    f32 = np.float32
    x = np.asarray(x, f32)
    y = np.asarray(target, f32)
    cc = np.asarray(cluster_centers, f32)
    filling_target = np.asarray(filling_target, f32)
    pt = np.asarray(prediction_target)

    nx = (x * x).sum(-1).astype(f32)
    ny = (y * y).sum(-1).astype(f32)
    ncc = (cc * cc).sum(-1).astype(f32)
    d_x = (nx[:, None] + ncc[None, :] - 2.0 * (x @ cc.T)).astype(f32)
    pred_x = d_x.argmin(1)

    idx_x = [np.where(pred_x == k)[0] for k in range(K)]
    idx_y = [np.where(pt == k)[0] for k in range(K)]
    nk = [len(i) for i in idx_x]
    mk = [len(i) for i in idx_y]
    S = _ceil128(max(max(nk), max(mk)))
    NB = S // 128
    OC = 6 * NB + 1

    # eps0 upper bounds per cost kind (exact max of C is not worth a launch)
    mx = np.sqrt(nx.max())
    my = np.sqrt(ny.max())
    eps0 = {"xy": max(f32(0.5 * (mx + my) ** 2), EPS),
            "xx": max(f32(0.5 * (2 * mx) ** 2), EPS),
            "yy": max(f32(0.5 * (2 * my) ** 2), EPS)}

    key = (S, NITER)
    if key not in _cache:
        nc = _build(S)
        _cache[key] = (nc, _make_runner(nc))
    nc, runner = _cache[key]

    t_arr = np.arange(NITER, dtype=f32)
    eps_seq = {}
    for kind, e0 in eps0.items():
        s = np.maximum(e0 * SCAL2 ** t_arr, EPS).astype(f32)
        eps_seq[kind] = np.concatenate([s, [EPS]]).astype(f32)
    kinds = ("xy", "xx", "yy")

    import ml_dtypes
    bf16 = ml_dtypes.bfloat16
    cpts = np.ascontiguousarray((-2.0 * cc.T).astype(bf16))
    cext = np.zeros((2, 8), f32)
    cext[0] = ncc
    cext[1] = -2.0

    in_maps = []
    host_terms = np.zeros(NCORES, f32)   # sum_p coeff * (aw.halfnx + bw.halfny)
    valid = np.zeros((NCORES, 3), f32)
    coeffs = np.array([1.0, -0.5, -0.5], f32)

    for k in range(K):
        xk = x[idx_x[k]]
        yk = y[idx_y[k]]
        cx, cy = nk[k], mk[k]
        nxk = nx[idx_x[k]]
        nyk = ny[idx_y[k]]

        def pts_tile(pts):
            t = np.zeros((64, S), bf16)
            t[:, :pts.shape[0]] = pts.T.astype(bf16)
            return t

        def ext_tile(n2):
            t = np.zeros((2, S), f32)
            t[0] = 1.0                     # h-row coefficient (all points)
            t[1, :n2.shape[0]] = -0.5 * n2
            return t

        xpts = pts_tile(xk)
        ypts = pts_tile(yk)
        xext = ext_tile(nxk)
        yext = ext_tile(nyk)

        lwx = f32(np.log(np.float64(1.0 / cx))) if cx else f32(0.0)
        lwy = f32(np.log(np.float64(1.0 / cy))) if cy else f32(0.0)
        # logw of the A (x/rows) and B (y/cols) side per problem
        lwA = (lwx, lwx, lwy)
        lwB = (lwy, lwx, lwy)

        inith = np.full((4, S), -BIG, f32)
        inith[0, :cy] = eps_seq["xy"][0] * lwy - 0.5 * nyk
        inith[1, :cx] = eps_seq["xx"][0] * lwx - 0.5 * nxk
        inith[2, :cy] = eps_seq["yy"][0] * lwy - 0.5 * nyk
        inith[3, :] = 1.0          # the constant ones row of the mov tiles

        hx = np.full(S, BIG, f32)
        hx[:cx] = 0.5 * nxk
        hy = np.full(S, BIG, f32)
        hy[:cy] = 0.5 * nyk
        awv = np.zeros(S, f32)
        if cx:
            awv[:cx] = f32(1.0 / cx)
        bwv = np.zeros(S, f32)
        if cy:
            bwv[:cy] = f32(1.0 / cy)

        bscal = np.zeros((1, 12 * NSEQ), f32)
        rtab = np.zeros((1, 3 * NSEQ), f32)
        for p, kind in enumerate(kinds):
            es = eps_seq[kind]
            bscal[0, p * NSEQ:(p + 1) * NSEQ] = 1.0 / es
            bscal[0, (3 + p) * NSEQ:(4 + p) * NSEQ] = -1.0 / es
            # tq tables: q += tq before the -eps_t-scaled writeback, so that
            # h' = eps_cons*logw - eps_t*q. f-side: cons = eps_t; g-side:
            # cons = eps_{t+1} (the f-half of the next iteration).
            bscal[0, (6 + p) * NSEQ:(7 + p) * NSEQ] = -lwA[p]
            tqg = np.zeros(NSEQ, f32)
            tqg[:NITER] = -(es[1:] / es[:NITER]) * lwB[p]
            bscal[0, (9 + p) * NSEQ:(10 + p) * NSEQ] = tqg
            rtab[0, p * NSEQ:(p + 1) * NSEQ] = -es

        in_maps.append({
            "xpts": xpts, "ypts": ypts, "xext": xext, "yext": yext,
            "inith": inith,
            "halfnx": _pk(hx, NB), "halfny": _pk(hy, NB),
            "aw": _pk(awv, NB), "bw": _pk(bwv, NB),
            "bscal": bscal, "rtab": rtab, "cpts": cpts, "cext": cext,
        })
        vk = f32(1.0) if (cx > 0 and cy > 0) else f32(0.0)
        valid[k] = vk
        ha = f32((awv * hx).sum(dtype=np.float64)) if cx else f32(0.0)
        hb = f32((bwv * hy).sum(dtype=np.float64)) if cy else f32(0.0)
        # per problem p: f-side host const uses A weights, g-side B weights
        hostA = (ha, ha, hb)
        hostB = (hb, ha, hb)
        # g2 consumes the t=NITER-1 f-half's h-row, whose logw bias used
        # eps_{NITER-1} instead of EPS; the resulting potential is uniformly
        # shifted by -(eps_{NITER-1}-EPS)*logw_A — add the exact shift back.
        delta = [float(eps_seq[kinds[p]][NITER - 1] - EPS) * float(lwA[p])
                 for p in range(3)]
        host_terms[k] = vk * float(
            sum(coeffs[p] * (hostA[p] + hostB[p] + delta[p])
                for p in range(3)))

    results = runner(in_maps)

    loss_med = np.float64(0.0)
    fill = np.zeros(8, np.float64)
    for k in range(K):
        o = results[k]["osum"].astype(np.float64)
        fill += nk[k] * o[0:8, OC - 1]
        for p in range(3):
            s_p = 0.0
            for side in range(2):
                q = p * 2 + side
                blk = o[0:2 * NB, q * NB:(q + 1) * NB]
                dln = sum(blk[b, b] for b in range(NB))
                dmp = sum(blk[NB + b, b] for b in range(NB))
                s_p += -float(EPS) * dln - dmp
            loss_med += valid[k, p] * coeffs[p] * s_p
        loss_med += host_terms[k]

    filling_x = (fill / N).astype(f32)
    loss_fil = np.mean((filling_x - filling_target) ** 2, dtype=f32)
    return np.asarray(f32(loss_fil + f32(loss_med)))


# revision 7
# speedup vs baseline: 1.2508x; 1.0305x over previous
"""Trainium2 Bass kernel for nn_LossKMeansWasserstein — single-launch design.

Architecture (v2): wall-clock in this axon-tunneled environment is dominated
by per-launch overhead (~0.35s stock, ~0.08s with a cached-jit launcher) and
host->device transfer (~50MB/s), not device compute (<1ms). So:

  1. ONE device launch per call (no cost-max prepass): eps0 per cost kind is
     replaced by the upper bound 0.5*(max|x|+max|y|)^2 (sim: ~2e-4 effect on
     the loss at NITER=22 vs the reference's exact-max schedule).
  2. Cluster k lives entirely on core k: its 3 Sinkhorn problems (xy, xx, yy)
     run interleaved for cross-problem engine pipelining.
  3. Uploads are compact (~235KB/core): point tiles go up as bf16 (the
     debiased divergence cancels the cost quantization — replica-verified
     ~2e-4 total), fp32 extras ride a tiny [2,S] tile, and per-(t,problem)
     eps scalars are broadcast to 128 partitions with one ones-matmul.
  4. The jitted PJRT launcher is built once and cached; per-call overhead is
     concat + dispatch only (per-call jax.jit in run_bass_kernel_spmd costs
     ~0.3s of retracing that this avoids).
  5. Activation-table placement is pinned to the exp+ln combined set
     (otherwise the greedy pass inserts ~276 Exp<->Ln table reloads).

Math: log-domain Sinkhorn on tilde-potentials G~ = g - 0.5|y|^2. Per
128-row block the PE accumulates a bf16 points matmul (x_i.y_j) plus an
fp32 rank-2 pass ([ones; -0.5|x|^2] x [h_j; ones]) into PSUM. Row-max on
DVE, fused exp+row-sum on ACT, then q = lnS + m'/eps (+ logw fold) is
transposed via PE and written back Copy-scaled by -eps_t as the next
half-update's h-row. Finals (f2/g2) skip the h-write and dot with the
cluster weights on the PE; the g2 half's stale logw bias is corrected
exactly on the host.
"""
import os
import sys
from contextlib import ExitStack

import numpy as np

sys.path.insert(0, "/opt/trn_rl_repo")

import concourse.bass as bass  # noqa: E402
import concourse.tile as tile  # noqa: E402
from concourse import bacc, mybir, bass2jax  # noqa: E402
from concourse.masks import make_identity  # noqa: E402


class _PinActTables:
    """Steer Bacc's activation-table placement to the one set that holds
    BOTH exp and ln ('natural_log_exp_and_others'): the greedy pass
    otherwise alternates exp-only/ln-only sets, inserting ~276 table
    reloads (~350us of ACT time). Only the placement pass sees the
    filtered view; emitted act_func_set_ids stay valid act_info indices,
    and the pinned set genuinely contains every function we use.
    """

    def __enter__(self):
        self._orig = bacc.get_activation_tables

        def filtered(arch):
            tabs = self._orig(arch)
            both = {mybir.ActivationFunctionType.Exp,
                    mybir.ActivationFunctionType.Ln}
            combined = "natural_log_exp_and_others"
            if not both <= tabs.get(combined, set()):
                return tabs            # unexpected act_info: leave untouched
            out = {}
            for name, funcs in tabs.items():
                if name != combined and both & funcs:
                    funcs = funcs - both
                out[name] = funcs
            return out

        bacc.get_activation_tables = filtered
        return self

    def __exit__(self, *exc):
        bacc.get_activation_tables = self._orig
        return False

import jax  # noqa: E402
from jax.sharding import Mesh, PartitionSpec  # noqa: E402

from jax.experimental.shard_map import shard_map as _sm  # noqa: E402


def _shard_map(f, mesh, in_specs, out_specs, check_rep):
    return _sm(f, mesh=mesh, in_specs=in_specs, out_specs=out_specs,
               check_rep=check_rep)

F32 = mybir.dt.float32
BF16 = mybir.dt.bfloat16
AF = mybir.ActivationFunctionType
ALU = mybir.AluOpType

N, M, D, K = 3072, 3072, 64, 8
BLUR = 0.05
EPS = np.float32(BLUR ** 2)
SCAL2 = np.float32(0.8 ** 2)
NITER = int(os.environ.get("KM_NITER", "22"))
NSEQ = NITER + 1
BIG = np.float32(1e7)
NCORES = 8

_cache = {}


def _ceil128(v):
    return max(128, ((v + 127) // 128) * 128)


# --------------------------------------------------------------------------
# device kernel
# --------------------------------------------------------------------------

def _build(S):
    NB = S // 128
    nc = bacc.Bacc("TRN2", target_bir_lowering=False, debug=False,
                   num_devices=NCORES)

    d = {}
    for name, shape, dt in (
        ("xpts", [64, S], BF16), ("ypts", [64, S], BF16),
        ("xext", [2, S], F32), ("yext", [2, S], F32),
        ("inith", [4, S], F32),
        ("halfnx", [128, NB], F32), ("halfny", [128, NB], F32),
        ("aw", [128, NB], F32), ("bw", [128, NB], F32),
        ("bscal", [1, 12 * NSEQ], F32), ("rtab", [1, 3 * NSEQ], F32),
        ("cpts", [64, 8], BF16), ("cext", [2, 8], F32),
    ):
        d[name] = nc.dram_tensor(name, shape, dt, kind="ExternalInput").ap()
    OC = 6 * NB + 1
    d_out = nc.dram_tensor("osum", [16, OC], F32, kind="ExternalOutput").ap()

    with tile.TileContext(nc) as tc, ExitStack() as ctx:
        cpool = ctx.enter_context(tc.tile_pool(name="cpool", bufs=1))
        g = {}
        for nm in ("xpts", "ypts", "xext", "yext", "inith", "halfnx",
                   "halfny", "aw", "bw", "bscal", "rtab", "cpts", "cext"):
            t = cpool.tile(list(d[nm].shape), d[nm].tensor.dtype,
                           tag=f"in_{nm}")
            nc.sync.dma_start(t[:], d[nm][:])
            g[nm] = t
        ident = cpool.tile([128, 128], F32, tag="ident")
        make_identity(nc, ident[:])
        g["ident"] = ident

        osum = cpool.tile([128, OC], F32)
        nc.vector.memset(osum[:], 0.0)

        # ---- dynamic-row tiles: row 0 = h (rewritten each half), row 1 = 1 ----
        dyns = {}
        for nm, hrow in (("dyn_b_xy", 0), ("dyn_a_xy", None),
                         ("dyn_b_xx", 1), ("dyn_a_xx", None),
                         ("dyn_b_yy", 2), ("dyn_a_yy", None)):
            dt_ = cpool.tile([2, S], F32, tag=nm)
            nc.sync.dma_start(dt_[1:2, :], g["inith"][3:4, :])
            if hrow is None:
                nc.vector.memset(dt_[0:1, :], 0.0)
            else:
                nc.sync.dma_start(dt_[0:1, :], g["inith"][hrow:hrow + 1, :])
            dyns[nm] = dt_

        # ---- broadcast per-(problem,t) scalars to 128 partitions ----
        onesrow = cpool.tile([1, 128], F32)
        nc.vector.memset(onesrow[:], 1.0)
        btab = cpool.tile([128, 12 * NSEQ], F32)
        with tc.tile_pool(name="setup_ps", bufs=1, space="PSUM") as sps:
            bc = sps.tile([128, 12 * NSEQ], F32, tag="bc")
            nc.tensor.matmul(bc[:], onesrow[:], g["bscal"][:])
            nc.scalar.copy(btab[:], bc[:])

            # ---- filling partial sums (independent of sinkhorn) ----
            fillps = sps.tile([8, 1], F32, tag="fillps")
            for b in range(NB):
                dxp = sps.tile([128, 8], F32, tag="dxp")
                nc.tensor.matmul(dxp[:], g["xpts"][:, b * 128:(b + 1) * 128],
                                 g["cpts"][:], start=True, stop=False)
                nc.tensor.matmul(dxp[:], g["xext"][:, b * 128:(b + 1) * 128],
                                 g["cext"][:], start=False, stop=True)
                mind = cpool.tile([128, 1], F32, tag="mind")
                nc.vector.tensor_reduce(mind[:], dxp[:], mybir.AxisListType.X,
                                        ALU.min)
                et = cpool.tile([128, 8], F32, tag="et")
                ssum = cpool.tile([128, 1], F32, tag="ssum")
                nc.scalar.activation(et[:], dxp[:], AF.Exp, bias=mind[:],
                                     scale=-1.0, accum_out=ssum[:])
                rs = cpool.tile([128, 1], F32, tag="rs")
                nc.vector.reciprocal(rs[:], ssum[:])
                soft = cpool.tile([128, 8], F32, tag="soft")
                nc.vector.tensor_scalar_mul(soft[:], et[:], rs[:])
                nc.tensor.matmul(fillps[:], soft[:], g["aw"][:, b:b + 1],
                                 start=(b == 0), stop=(b == NB - 1))
            nc.scalar.copy(osum[0:8, OC - 1:OC], fillps[:])

        # ---- the 3 sinkhorn problems, interleaved ----
        probs = [
            dict(pi=0, ptsA=g["xpts"], ptsB=g["ypts"],
                 extA=g["xext"], extB=g["yext"],
                 dynA=dyns["dyn_a_xy"], dynB=dyns["dyn_b_xy"],
                 hA=g["halfnx"], hB=g["halfny"], wA=g["aw"], wB=g["bw"]),
            dict(pi=1, ptsA=g["xpts"], ptsB=g["xpts"],
                 extA=g["xext"], extB=g["xext"],
                 dynA=dyns["dyn_a_xx"], dynB=dyns["dyn_b_xx"],
                 hA=g["halfnx"], hB=g["halfnx"], wA=g["aw"], wB=g["aw"]),
            dict(pi=2, ptsA=g["ypts"], ptsB=g["ypts"],
                 extA=g["yext"], extB=g["yext"],
                 dynA=dyns["dyn_a_yy"], dynB=dyns["dyn_b_yy"],
                 hA=g["halfny"], hB=g["halfny"], wA=g["bw"], wB=g["bw"]),
        ]

        psv = ctx.enter_context(tc.tile_pool(name="psv", bufs=2, space="PSUM"))
        psq = ctx.enter_context(tc.tile_pool(name="psq", bufs=1, space="PSUM"))
        wpool = ctx.enter_context(tc.tile_pool(name="wpool", bufs=2))
        epool = ctx.enter_context(tc.tile_pool(name="epool", bufs=1))

        def half(pr, t, fside, final):
            pi = pr["pi"]
            if fside:
                ptsS, ptsM = pr["ptsA"], pr["ptsB"]
                extS, dyn_in, dyn_out = pr["extA"], pr["dynB"], pr["dynA"]
                halfn, w = pr["hA"], pr["wA"]
                # h' consumed by the g-half of the SAME iteration t
                tq_off = (6 + pi) * NSEQ + t        # -logw_A
            else:
                ptsS, ptsM = pr["ptsB"], pr["ptsA"]
                extS, dyn_in, dyn_out = pr["extB"], pr["dynA"], pr["dynB"]
                halfn, w = pr["hB"], pr["wB"]
                # h' consumed by the f-half of iteration t+1 (incl. final)
                tq_off = (9 + pi) * NSEQ + t        # -(eps_{t+1}/eps_t)*logw_B
            inveps = g["btab_view"][:, pi * NSEQ + t:pi * NSEQ + t + 1]
            nginveps = g["btab_view"][:, (3 + pi) * NSEQ + t:
                                      (3 + pi) * NSEQ + t + 1]
            ne_off = pi * NSEQ + t                  # -eps_t

            sd = "f" if fside else "g"
            lnm = wpool.tile([128, 2 * NB], F32, tag=f"lnm{pi}{sd}")
            sv = wpool.tile([128, NB], F32, tag=f"sv{pi}{sd}")
            for b in range(NB):
                vps = psv.tile([128, S], F32, tag="vps")
                for c0 in range(0, S, 512):
                    c1 = min(c0 + 512, S)
                    nc.tensor.matmul(vps[:, c0:c1],
                                     ptsS[:, b * 128:(b + 1) * 128],
                                     ptsM[:, c0:c1], start=True, stop=False)
                    nc.tensor.matmul(vps[:, c0:c1],
                                     extS[:, b * 128:(b + 1) * 128],
                                     dyn_in[:, c0:c1], start=False, stop=True)
                nc.vector.tensor_reduce(lnm[:, NB + b:NB + b + 1], vps[:],
                                        mybir.AxisListType.X, ALU.max)
                bv = wpool.tile([128, 1], F32, tag=f"bv{pi}")
                nc.vector.tensor_scalar_mul(bv[:], lnm[:, NB + b:NB + b + 1],
                                            nginveps)
                expo = epool.tile([128, S], F32, tag=f"expo{pi}")
                nc.scalar.activation(expo[:], vps[:], AF.Exp, bias=bv[:],
                                     scale=inveps, accum_out=sv[:, b:b + 1])
            nc.scalar.activation(lnm[:, 0:NB], sv[:], AF.Ln)
            nc.vector.tensor_add(lnm[:, NB:2 * NB], lnm[:, NB:2 * NB],
                                 halfn[:])
            if final:
                q = pi * 2 + (0 if fside else 1)
                dps = psq.tile([2 * NB, NB], F32, tag="dot")
                nc.tensor.matmul(dps[:], lnm[:], w[:])
                nc.scalar.copy(osum[0:2 * NB, q * NB:(q + 1) * NB], dps[:])
            else:
                tq = g["btab_view"][:, tq_off:tq_off + 1]
                qv = wpool.tile([128, NB], F32, tag=f"qv{pi}")
                nc.vector.tensor_scalar_mul(qv[:], lnm[:, NB:2 * NB], inveps)
                nc.vector.tensor_add(qv[:], qv[:], lnm[:, 0:NB])
                nc.vector.tensor_scalar_add(qv[:], qv[:], tq)
                qT = psq.tile([1, S], F32, tag="qT")
                for b in range(NB):
                    nc.tensor.matmul(qT[0:1, b * 128:(b + 1) * 128],
                                     qv[:, b:b + 1], g["ident"][:])
                nc.scalar.activation(
                    dyn_out[0:1, :], qT[:], AF.Copy, bias=0.0,
                    scale=g["rtab"][0:1, ne_off:ne_off + 1])

        g["btab_view"] = btab
        for t in range(NITER):
            for pr in probs:
                half(pr, t, True, False)
            for pr in probs:
                half(pr, t, False, False)
        for pr in probs:
            half(pr, NITER, True, True)
        for pr in probs:
            half(pr, NITER, False, True)

        nc.sync.dma_start(d_out[:], osum[0:16, :])
    with _PinActTables():
        nc.compile()
    return nc


# --------------------------------------------------------------------------
# cached-jit PJRT launcher (per-call jax.jit in run_bass_kernel_spmd costs
# ~0.3s of retracing; build the jitted callable once instead)
# --------------------------------------------------------------------------

def _make_runner(nc):
    bass2jax.install_neuronx_cc_hook()
    partition_name = (nc.partition_id_tensor.name
                      if nc.partition_id_tensor else None)
    in_names, out_names, out_avals, zero_shapes = [], [], [], []
    for alloc in nc.m.functions[0].allocations:
        if not isinstance(alloc, mybir.MemoryLocationSet):
            continue
        name = alloc.memorylocations[0].name
        if alloc.kind == "ExternalInput":
            if name != partition_name:
                in_names.append(name)
        elif alloc.kind == "ExternalOutput":
            shape = tuple(alloc.tensor_shape)
            dtype = mybir.dt.np(alloc.dtype)
            out_names.append(name)
            out_avals.append(jax.core.ShapedArray(shape, dtype))
            zero_shapes.append((shape, dtype))
    n_params = len(in_names)
    n_outs = len(out_avals)
    in_names_all = list(in_names) + list(out_names)
    if partition_name is not None:
        in_names_all.append(partition_name)
    donate = tuple(range(n_params, n_params + n_outs))

    def _body(*args):
        operands = list(args)
        if partition_name is not None:
            operands.append(bass2jax.partition_id_tensor())
        outs = bass2jax._bass_exec_p.bind(
            *operands, out_avals=tuple(out_avals),
            in_names=tuple(in_names_all), out_names=tuple(out_names),
            lowering_input_output_aliases=(), sim_require_finite=True,
            sim_require_nnan=True, nc=nc)
        return tuple(outs)

    devices = jax.devices()[:NCORES]
    mesh = Mesh(np.asarray(devices), ("core",))
    in_specs = (PartitionSpec("core"),) * (n_params + n_outs)
    out_specs = (PartitionSpec("core"),) * n_outs
    sharded = jax.jit(
        _shard_map(_body, mesh, in_specs, out_specs, False),
        donate_argnums=donate, keep_unused=True)

    def run(in_maps):
        concat_in = [
            np.concatenate([np.asarray(in_maps[c][nm]) for c in range(NCORES)],
                           axis=0)
            for nm in in_names]
        concat_zeros = [np.zeros((NCORES * s[0], *s[1:]), dt)
                        for s, dt in zero_shapes]
        out_arrs = sharded(*concat_in, *concat_zeros)
        return [
            {nm: np.asarray(out_arrs[i]).reshape(NCORES, *out_avals[i].shape)[c]
             for i, nm in enumerate(out_names)}
            for c in range(NCORES)]

    return run


# --------------------------------------------------------------------------
# host orchestration
# --------------------------------------------------------------------------

def _pk(vec, nb):
    """[nb*128] -> [128, nb]; column b holds points b*128..b*128+127."""
    return np.ascontiguousarray(vec.reshape(nb, 128).T)


def kernel(x, target, cluster_centers, filling_target, prediction_target):
    f32 = np.float32
    x = np.asarray(x, f32)
    y = np.asarray(target, f32)
    cc = np.asarray(cluster_centers, f32)
    filling_target = np.asarray(filling_target, f32)
    pt = np.asarray(prediction_target)

    nx = (x * x).sum(-1).astype(f32)
    ny = (y * y).sum(-1).astype(f32)
    ncc = (cc * cc).sum(-1).astype(f32)
    d_x = (nx[:, None] + ncc[None, :] - 2.0 * (x @ cc.T)).astype(f32)
    pred_x = d_x.argmin(1)

    idx_x = [np.where(pred_x == k)[0] for k in range(K)]
    idx_y = [np.where(pt == k)[0] for k in range(K)]
    nk = [len(i) for i in idx_x]
    mk = [len(i) for i in idx_y]
    S = _ceil128(max(max(nk), max(mk)))
    NB = S // 128
    OC = 6 * NB + 1

    # eps0 upper bounds per cost kind (exact max of C is not worth a launch)
    mx = np.sqrt(nx.max())
    my = np.sqrt(ny.max())
    eps0 = {"xy": max(f32(0.5 * (mx + my) ** 2), EPS),
            "xx": max(f32(0.5 * (2 * mx) ** 2), EPS),
            "yy": max(f32(0.5 * (2 * my) ** 2), EPS)}

    key = (S, NITER)
    if key not in _cache:
        nc = _build(S)
        _cache[key] = (nc, _make_runner(nc))
    nc, runner = _cache[key]

    t_arr = np.arange(NITER, dtype=f32)
    eps_seq = {}
    for kind, e0 in eps0.items():
        s = np.maximum(e0 * SCAL2 ** t_arr, EPS).astype(f32)
        eps_seq[kind] = np.concatenate([s, [EPS]]).astype(f32)
    kinds = ("xy", "xx", "yy")

    import ml_dtypes
    bf16 = ml_dtypes.bfloat16
    cpts = np.ascontiguousarray((-2.0 * cc.T).astype(bf16))
    cext = np.zeros((2, 8), f32)
    cext[0] = ncc
    cext[1] = -2.0

    in_maps = []
    host_terms = np.zeros(NCORES, f32)   # sum_p coeff * (aw.halfnx + bw.halfny)
    valid = np.zeros((NCORES, 3), f32)
    coeffs = np.array([1.0, -0.5, -0.5], f32)

    for k in range(K):
        xk = x[idx_x[k]]
        yk = y[idx_y[k]]
        cx, cy = nk[k], mk[k]
        nxk = nx[idx_x[k]]
        nyk = ny[idx_y[k]]

        def pts_tile(pts):
            t = np.zeros((64, S), bf16)
            t[:, :pts.shape[0]] = pts.T.astype(bf16)
            return t

        def ext_tile(n2):
            t = np.zeros((2, S), f32)
            t[0] = 1.0                     # h-row coefficient (all points)
            t[1, :n2.shape[0]] = -0.5 * n2
            return t

        xpts = pts_tile(xk)
        ypts = pts_tile(yk)
        xext = ext_tile(nxk)
        yext = ext_tile(nyk)

        lwx = f32(np.log(np.float64(1.0 / cx))) if cx else f32(0.0)
        lwy = f32(np.log(np.float64(1.0 / cy))) if cy else f32(0.0)
        # logw of the A (x/rows) and B (y/cols) side per problem
        lwA = (lwx, lwx, lwy)
        lwB = (lwy, lwx, lwy)

        inith = np.full((4, S), -BIG, f32)
        inith[0, :cy] = eps_seq["xy"][0] * lwy - 0.5 * nyk
        inith[1, :cx] = eps_seq["xx"][0] * lwx - 0.5 * nxk
        inith[2, :cy] = eps_seq["yy"][0] * lwy - 0.5 * nyk
        inith[3, :] = 1.0          # the constant ones row of the mov tiles

        hx = np.full(S, BIG, f32)
        hx[:cx] = 0.5 * nxk
        hy = np.full(S, BIG, f32)
        hy[:cy] = 0.5 * nyk
        awv = np.zeros(S, f32)
        if cx:
            awv[:cx] = f32(1.0 / cx)
        bwv = np.zeros(S, f32)
        if cy:
            bwv[:cy] = f32(1.0 / cy)

        bscal = np.zeros((1, 12 * NSEQ), f32)
        rtab = np.zeros((1, 3 * NSEQ), f32)
        for p, kind in enumerate(kinds):
            es = eps_seq[kind]
            bscal[0, p * NSEQ:(p + 1) * NSEQ] = 1.0 / es
            bscal[0, (3 + p) * NSEQ:(4 + p) * NSEQ] = -1.0 / es
            # tq tables: q += tq before the -eps_t-scaled writeback, so that
            # h' = eps_cons*logw - eps_t*q. f-side: cons = eps_t; g-side:
            # cons = eps_{t+1} (the f-half of the next iteration).
            bscal[0, (6 + p) * NSEQ:(7 + p) * NSEQ] = -lwA[p]
            tqg = np.zeros(NSEQ, f32)
            tqg[:NITER] = -(es[1:] / es[:NITER]) * lwB[p]
            bscal[0, (9 + p) * NSEQ:(10 + p) * NSEQ] = tqg
            rtab[0, p * NSEQ:(p + 1) * NSEQ] = -es

        in_maps.append({
            "xpts": xpts, "ypts": ypts, "xext": xext, "yext": yext,
            "inith": inith,
            "halfnx": _pk(hx, NB), "halfny": _pk(hy, NB),
            "aw": _pk(awv, NB), "bw": _pk(bwv, NB),
            "bscal": bscal, "rtab": rtab, "cpts": cpts, "cext": cext,
        })
        vk = f32(1.0) if (cx > 0 and cy > 0) else f32(0.0)
        valid[k] = vk
        ha = f32((awv * hx).sum(dtype=np.float64)) if cx else f32(0.0)
        hb = f32((bwv * hy).sum(dtype=np.float64)) if cy else f32(0.0)
        # per problem p: f-side host const uses A weights, g-side B weights
        hostA = (ha, ha, hb)
        hostB = (hb, ha, hb)
        # g2 consumes the t=NITER-1 f-half's h-row, whose logw bias used
        # eps_{NITER-1} instead of EPS; the resulting potential is uniformly
        # shifted by -(eps_{NITER-1}-EPS)*logw_A — add the exact shift back.
        delta = [float(eps_seq[kinds[p]][NITER - 1] - EPS) * float(lwA[p])
                 for p in range(3)]
        host_terms[k] = vk * float(
            sum(coeffs[p] * (hostA[p] + hostB[p] + delta[p])
                for p in range(3)))

    results = runner(in_maps)

    loss_med = np.float64(0.0)
    fill = np.zeros(8, np.float64)
    for k in range(K):
        o = results[k]["osum"].astype(np.float64)
        fill += nk[k] * o[0:8, OC - 1]
        for p in range(3):
            s_p = 0.0
            for side in range(2):
                q = p * 2 + side
                blk = o[0:2 * NB, q * NB:(q + 1) * NB]
                dln = sum(blk[b, b] for b in range(NB))
                dmp = sum(blk[NB + b, b] for b in range(NB))
                s_p += -float(EPS) * dln - dmp
            loss_med += valid[k, p] * coeffs[p] * s_p
        loss_med += host_terms[k]

    filling_x = (fill / N).astype(f32)
    loss_fil = np.mean((filling_x - filling_target) ** 2, dtype=f32)
    return np.asarray(f32(loss_fil + f32(loss_med)))
